# revision 1
# baseline (speedup 1.0000x reference)
"""Causal multi-head attention TRN2 kernel (8 NeuronCores).

Problem: B=4, S=2048, D=1024, H=16 heads, head_dim=64 (fp32 reference).

Sharding: data-parallel over batch (4) x tensor-parallel over head-groups (2).
Core c handles batch c//2 with heads (c%2)*8 .. (c%2)*8+8 and produces a
partial [S, D] output (its head-group's contribution to the O-projection,
without b_o). Host sums the two partials per batch and adds b_o.

Single fused pipeline tuned to keep the PE array continuously streaming
(TRN2 PE p-state drops to 1.2/0.65 GHz after any idle gap, recovering only
after 3us of continuous work):
  - attention is software-pipelined: scores(t+1) is emitted before wv(t) so
    the PE never waits for the ACT exp;
  - qk-projection chains of pair j+1 are sprinkled inside pair j's attention
    as filler PE work while the ACT engine catches up on exp;
  - output-projection s-tiles are interleaved into pair 3's attention;
  - wv PSUM banks are drained to SBUF staging immediately; the softmax
    normalization (reciprocal + DRAM-bounce partition broadcast) happens off
    the critical path;
  - input DRAM layouts are DMA-priority-ordered (block-major x, pair-major
    W_q/W_k) so the first projection chain starts ~5us in.
"""

import math

import numpy as np

B, S, D, H = 4, 2048, 1024, 16
HD = D // H        # 64
NCORES = 8
HPC = H // 2       # heads per core: 8
NPAIR = HPC // 2   # head pairs per core: 4
KT = D // 128      # contraction tiles: 8
ST = S // 128      # seq tiles of 128: 16
SB = S // 512      # seq blocks of 512: 4

_BUILT = {}
LAST_RESULTS = None  # BassKernelResults of the most recent run (for test.py)


def _build_nc():
    import concourse.bass as bass
    import concourse.mybir as mybir
    from concourse import tile

    f32 = mybir.dt.float32
    bf16 = mybir.dt.bfloat16
    AF = mybir.ActivationFunctionType
    OP = mybir.AluOpType

    nc = bass.Bass("TRN2", target_bir_lowering=False, debug=False,
                   num_devices=NCORES)

    # DRAM layouts are pre-arranged on the host to match SBUF tiles exactly.
    # xT: col = c*4096 + k*512 + sc  (block-major so block 0 lands first)
    # wq/wk: col = j*1024 + k*128 + e  (pair-major so pair 0 lands first)
    xT_d = nc.dram_tensor("xT", [128, SB * KT * 512], bf16,
                          kind="ExternalInput").ap()
    wq_d = nc.dram_tensor("wq", [128, NPAIR * KT * 128], bf16,
                          kind="ExternalInput").ap()
    wk_d = nc.dram_tensor("wk", [128, NPAIR * KT * 128], bf16,
                          kind="ExternalInput").ap()
    wv_d = nc.dram_tensor("wv", [128, KT * 512], bf16,
                          kind="ExternalInput").ap()
    wo_d = nc.dram_tensor("wo", [128, NPAIR * D], bf16,
                          kind="ExternalInput").ap()
    bq_d = nc.dram_tensor("bq", [128, NPAIR], f32, kind="ExternalInput").ap()
    bk_d = nc.dram_tensor("bk", [128, NPAIR], f32, kind="ExternalInput").ap()
    bv_d = nc.dram_tensor("bv", [128, 512], f32, kind="ExternalInput").ap()
    mask_d = nc.dram_tensor("mask", [128, 512], bf16,
                            kind="ExternalInput").ap()
    out_d = nc.dram_tensor("out", [S, D], f32, kind="ExternalOutput").ap()

    with tile.TileContext(nc) as tc:
        with tc.tile_pool(name="persist", bufs=1) as pp, \
             tc.tile_pool(name="pacc", bufs=2, space="PSUM") as pacc, \
             tc.tile_pool(name="pss", bufs=2, space="PSUM") as pssp, \
             tc.tile_pool(name="pwv", bufs=1, space="PSUM") as pwvp, \
             tc.tile_pool(name="attn", bufs=6) as attnp, \
             tc.tile_pool(name="norm", bufs=3) as normp, \
             tc.tile_pool(name="ost", bufs=3) as ostp, \
             tc.tile_pool(name="scr", bufs=4, space="DRAM") as scrp:

            xt_all = pp.tile([128, SB * KT * 512], bf16, tag="xt", name="xt")
            wq_all = pp.tile([128, NPAIR * KT * 128], bf16, tag="wq", name="wq")
            wk_all = pp.tile([128, NPAIR * KT * 128], bf16, tag="wk", name="wk")
            wv_all = pp.tile([128, KT * 512], bf16, tag="wv", name="wv")
            wo_all = pp.tile([128, NPAIR * D], bf16, tag="wo", name="wo")
            bq = pp.tile([128, NPAIR], f32, tag="bq", name="bq")
            bk = pp.tile([128, NPAIR], f32, tag="bk", name="bk")
            bv = pp.tile([128, 512], f32, tag="bv", name="bv")
            mask = pp.tile([128, 512], bf16, tag="mask", name="mask")
            qT = [pp.tile([128, S], bf16, tag=f"qT{j}", name=f"qT{j}") for j in range(NPAIR)]
            kTt = [pp.tile([128, S], bf16, tag=f"kT{j}", name=f"kT{j}") for j in range(NPAIR)]
            v_all = [pp.tile([128, 8 * 65], bf16, tag=f"v{t}", name=f"v{t}")
                     for t in range(ST)]
            wvT = [pp.tile([128, S], bf16, tag=f"wvT{j}", name=f"wvT{j}") for j in range(NPAIR)]

            # ---- input DMAs ----
            # startup is HBM-bandwidth-bound: the three issue streams each
            # get ~1/3 of fabric bandwidth, so spread the early-needed
            # tensors (wq0/wk0/mask/biases, x c0+c1, wv) evenly across all
            # three streams and defer the heavies (x c2/c3, pair-1..3
            # weights, wo) behind them.
            # fine-grained early chunks: each issue stream keeps <=3 DMAs in
            # flight, so smaller chunks make the in-flight window track the
            # need-order (c0 strictly before c1 before wv) more closely
            nc.scalar.dma_start(wq_all[:, 0:1024], wq_d[:, 0:1024])
            nc.scalar.dma_start(mask, mask_d[:, :])
            nc.scalar.dma_start(wk_all[:, 0:1024], wk_d[:, 0:1024])
            for i in range(3):
                cs = slice(i * 512, (i + 1) * 512)
                nc.sync.dma_start(xt_all[:, cs], xT_d[:, cs])
            for i in range(3, 6):
                cs = slice(i * 512, (i + 1) * 512)
                nc.gpsimd.dma_start(xt_all[:, cs], xT_d[:, cs])
            nc.scalar.dma_start(xt_all[:, 3072:4096], xT_d[:, 3072:4096])
            nc.scalar.dma_start(bq, bq_d[:, :])
            nc.scalar.dma_start(bk, bk_d[:, :])
            nc.scalar.dma_start(bv, bv_d[:, :])
            for i in range(8, 10):
                cs = slice(i * 512, (i + 1) * 512)
                nc.sync.dma_start(xt_all[:, cs], xT_d[:, cs])
            for i in range(10, 12):
                cs = slice(i * 512, (i + 1) * 512)
                nc.gpsimd.dma_start(xt_all[:, cs], xT_d[:, cs])
            nc.scalar.dma_start(xt_all[:, 6144:7168], xT_d[:, 6144:7168])
            nc.scalar.dma_start(xt_all[:, 7168:8192], xT_d[:, 7168:8192])
            nc.sync.dma_start(wv_all[:, 0:1024], wv_d[:, 0:1024])
            nc.gpsimd.dma_start(wv_all[:, 1024:2048], wv_d[:, 1024:2048])
            nc.sync.dma_start(wv_all[:, 2048:3072], wv_d[:, 2048:3072])
            nc.gpsimd.dma_start(wv_all[:, 3072:4096], wv_d[:, 3072:4096])
            for c in range(2, SB):
                cs0 = slice(c * 4096, c * 4096 + 2048)
                cs1 = slice(c * 4096 + 2048, (c + 1) * 4096)
                nc.sync.dma_start(xt_all[:, cs0], xT_d[:, cs0])
                nc.gpsimd.dma_start(xt_all[:, cs1], xT_d[:, cs1])
            nc.scalar.dma_start(wq_all[:, 1024:], wq_d[:, 1024:])
            nc.scalar.dma_start(wk_all[:, 1024:], wk_d[:, 1024:])
            nc.scalar.dma_start(wo_all, wo_d[:, :])

            # prewarm the ACT exp table while DMAs stream (first activation
            # triggers a 1.3us ACT_TABLE_LOAD; don't pay it on the first
            # real softmax tile)
            warm = pp.tile([128, 1], f32, tag="warm", name="warm")
            nc.gpsimd.memset(warm[:, :], 0.0)
            nc.scalar.activation(warm[:, :], warm[:, :], AF.Exp, scale=1.0)

            def xt(k, c):
                return xt_all[:, c * 4096 + k * 512:c * 4096 + (k + 1) * 512]

            wvk = [wv_all[:, 512 * k:512 * (k + 1)] for k in range(KT)]
            wo = [wo_all[:, D * j:D * (j + 1)] for j in range(NPAIR)]

            def emit_qk_chain(j, c, which):
                cs = slice(512 * c, 512 * c + 512)
                ps = pacc.tile([128, 512], f32, tag="acc", name="acc")
                w_all = wq_all if which == 'q' else wk_all
                dst = qT[j] if which == 'q' else kTt[j]
                bias = bq if which == 'q' else bk
                for k in range(KT):
                    nc.tensor.matmul(
                        ps[:, :],
                        lhsT=w_all[:, j * 1024 + k * 128:j * 1024 + (k + 1) * 128],
                        rhs=xt(k, c),
                        start=(k == 0), stop=(k == KT - 1))
                nc.vector.tensor_scalar_add(dst[:, cs], ps[:, :],
                                            bias[:, j:j + 1])

            def emit_v(t):
                ps = pacc.tile([128, 512], f32, tag="acc", name="acc")
                c, i = t // 4, t % 4
                for k in range(KT):
                    nc.tensor.matmul(
                        ps[:, :],
                        lhsT=xt(k, c)[:, 128 * i:128 * i + 128],
                        rhs=wvk[k],
                        start=(k == 0), stop=(k == KT - 1))
                nc.gpsimd.memset(
                    v_all[t].rearrange("p (h e) -> p h e", e=65)[:, :, 64:65],
                    1.0)
                nc.vector.tensor_tensor(
                    v_all[t].rearrange("p (h e) -> p h e", e=65)[:, :, 0:64],
                    ps.rearrange("p (h e) -> p h e", e=64),
                    bv.rearrange("p (h e) -> p h e", e=64),
                    op=OP.add)

            finishers = []

            def emit_attn_block(j, b, fillers, fill_every):
                nt = 4 * b + 4
                wvA = pwvp.tile([65, 512], f32, tag="wvA", name="wvA")
                wvB = pwvp.tile([65, 512], f32, tag="wvB", name="wvB")
                live = {}

                def scores(t):
                    off = max(0, 128 * t - 512 * b)
                    qs = slice(512 * b + off, 512 * b + 512)
                    ts = slice(128 * t, 128 * t + 128)
                    ps = pssp.tile([128, 1024], f32, tag="pss", name="pss")
                    nc.tensor.matmul(
                        ps[:, off:512], lhsT=kTt[j][0:64, ts],
                        rhs=qT[j][0:64, qs], start=True, stop=True,
                        tile_position=(0, 0))
                    nc.tensor.matmul(
                        ps[:, 512 + off:1024], lhsT=kTt[j][64:128, ts],
                        rhs=qT[j][64:128, qs], start=True, stop=True,
                        tile_position=(64, 0))
                    live[t] = (ps, attnp.tile([128, 1024], bf16, tag="at", name="at"),
                               off)

                def expmask(t):
                    ps, at, off = live[t]
                    if off:
                        nc.scalar.activation(
                            at.rearrange("p (h w) -> p h w", h=2)[:, :, off:512],
                            ps.rearrange("p (h w) -> p h w", h=2)[:, :, off:512],
                            AF.Exp, scale=0.125)
                    else:
                        nc.scalar.activation(at[:, :], ps[:, :], AF.Exp,
                                             scale=0.125)
                    if t >= 4 * b:
                        w = 512 - off
                        atw = at.rearrange("p (h w) -> p h w",
                                           h=2)[:, :, off:512]
                        msl = mask[:, None, 0:w].broadcast_to((128, 2, w))
                        nc.vector.tensor_tensor(atw, atw, msl, op=OP.mult)

                def wv(t):
                    ps, at, off = live.pop(t)
                    nc.tensor.matmul(
                        wvA[:, off:512],
                        lhsT=v_all[t][:, 130 * j:130 * j + 65],
                        rhs=at[:, off:512],
                        start=(t == 0), stop=(t == nt - 1))
                    nc.tensor.matmul(
                        wvB[:, off:512],
                        lhsT=v_all[t][:, 130 * j + 65:130 * j + 130],
                        rhs=at[:, 512 + off:1024],
                        start=(t == 0), stop=(t == nt - 1))

                scores(0)
                expmask(0)
                for t in range(1, nt):
                    scores(t)
                    wv(t - 1)
                    expmask(t)
                    if t == 2 and finishers:
                        finishers.pop(0)()
                    if t % fill_every == 0 and fillers:
                        fillers.pop(0)()
                wv(nt - 1)

                # drain wv PSUM immediately (frees the banks for the next
                # block) and kick off the denominator redistribute; the
                # DVE reciprocal is deferred into the NEXT block (when the
                # DMA has landed) so it never stalls the in-order DVE
                # queue ahead of that block's masks, and the final
                # multiplies run on gpsimd where the rsb wait blocks
                # nothing critical.
                bs = slice(512 * b, 512 * b + 512)
                stg = normp.tile([65, 1024], f32, tag="stg", name="stg")
                nc.vector.tensor_copy(stg[:, 0:512], wvA[:, :])
                nc.vector.tensor_copy(stg[:, 512:1024], wvB[:, :])
                sumsq = normp.tile([128, 8], f32, tag="sumsq", name="sumsq")
                nc.sync.dma_start(sumsq[:, :], stg[64:65, :])

                def finish(j=j, b=b, stg=stg, sumsq=sumsq, bs=bs):
                    rq = normp.tile([128, 8], f32, tag="rq", name="rq")
                    nc.vector.reciprocal(rq[:, :], sumsq[:, :])
                    scr = scrp.tile([1, 1024], f32, tag="scr", name="scr")
                    nc.sync.dma_start(scr[:, :], rq[:, :])
                    rsb = normp.tile([64, 1024], f32, tag="rsb", name="rsb")
                    nc.sync.dma_start(rsb[:, :],
                                      scr[0:1, :].broadcast_to((64, 1024)))
                    nc.gpsimd.tensor_tensor(wvT[j][0:64, bs],
                                            stg[0:64, 0:512],
                                            rsb[:, 0:512], op=OP.mult)
                    nc.gpsimd.tensor_tensor(wvT[j][64:128, bs],
                                            stg[0:64, 512:1024],
                                            rsb[:, 512:1024], op=OP.mult)
                finishers.append(finish)

            def emit_oproj_stile(s):
                ss = slice(128 * s, 128 * s + 128)
                ost = ostp.tile([128, 1024], f32, tag="ost", name="ost")
                for n in range(2):
                    ns = slice(512 * n, 512 * n + 512)
                    ps = pacc.tile([128, 512], f32, tag="acc", name="acc")
                    for j in range(NPAIR):
                        nc.tensor.matmul(
                            ps[:, :], lhsT=wvT[j][:, ss], rhs=wo[j][:, ns],
                            start=(j == 0), stop=(j == NPAIR - 1))
                    nc.vector.tensor_copy(ost[:, ns], ps[:, :])
                nc.scalar.dma_start(out_d[ss, :], ost[:, :])

            # ---- emission schedule ----
            # pair 0 is emitted c-incrementally so PE work only ever needs
            # the x chunks that have already streamed in; its attention
            # blocks 0-1 use the interleaved v/qk chains as natural filler
            f1 = [(lambda j=1, c=c, w=w: emit_qk_chain(j, c, w))
                  for c in range(SB) for w in ('q', 'k')]
            emit_qk_chain(0, 0, 'q')
            emit_qk_chain(0, 0, 'k')
            emit_qk_chain(0, 1, 'q')
            emit_qk_chain(0, 1, 'k')
            for t in range(4):
                emit_v(t)
            emit_attn_block(0, 0, [], 3)
            emit_qk_chain(0, 2, 'q')
            emit_qk_chain(0, 2, 'k')
            for t in range(4, 8):
                emit_v(t)
            emit_attn_block(0, 1, [], 3)
            emit_qk_chain(0, 3, 'q')
            emit_qk_chain(0, 3, 'k')
            for t in range(8, 12):
                emit_v(t)
            emit_attn_block(0, 2, f1, 3)
            for t in range(12, 16):
                emit_v(t)
            emit_attn_block(0, 3, f1, 3)
            while f1:
                f1.pop(0)()

            f2 = [(lambda j=2, c=c, w=w: emit_qk_chain(j, c, w))
                  for c in range(SB) for w in ('q', 'k')]
            for b in range(SB):
                emit_attn_block(1, b, f2, 3)
            while f2:
                f2.pop(0)()

            # attn(2) fillers: qk(3) chains c0-c2q; the last two qk(3) chains
            # are held back as the only norm-independent fillers attn(3) has
            # before its own first norm completes
            f3 = [(lambda j=3, c=c, w=w: emit_qk_chain(j, c, w))
                  for c in range(3) for w in ('q', 'k')]
            for b in range(SB):
                emit_attn_block(2, b, f3, 3)
            while f3:
                f3.pop(0)()

            # oproj s-tiles for block b become fillers only at block b+2 so
            # a popped filler never waits on an in-flight norm chain; the
            # two held-back blocks drain at the end, where oproj(8..11)
            # covers the final norm chain's latency
            f4 = [(lambda w=w: emit_qk_chain(3, 3, w)) for w in ('q', 'k')]
            for b in range(SB):
                if b >= 2:
                    f4.extend([(lambda s=s: emit_oproj_stile(s))
                               for s in range(4 * (b - 2), 4 * (b - 2) + 4)])
                emit_attn_block(3, b, f4, 3)
            while finishers:
                finishers.pop(0)()
            f4.extend([(lambda s=s: emit_oproj_stile(s))
                       for s in range(8, 16)])
            while f4:
                f4.pop(0)()

    _split_excess_waits(nc, limit=1)
    return nc


def _split_excess_waits(nc, limit=1):
    """This container's walrus encodes at most one sem wait per instruction;
    move excess waits onto standalone EventSemaphore ops just before each
    over-limit instruction (same engine stream, so semantics preserved)."""
    import concourse.mybir as mybir
    n = 0
    for fn in nc.m.functions:
        for bb in fn.blocks:
            new_insts = []
            for inst in bb.instructions:
                si = inst.sync_info
                if si is not None and si.on_wait and len(si.on_wait) > limit:
                    waits = list(si.on_wait)
                    for i, w in enumerate(waits[limit:]):
                        wi = mybir.InstEventSemaphore(
                            name=f"{inst.name}-wsplit{i}", ins=[], outs=[],
                            sync_info=mybir.SyncInfo(on_wait=[w], on_update=[]))
                        wi.engine = inst.engine
                        nc.register_instruction(wi)
                        new_insts.append(wi)
                        n += 1
                    si.on_wait = waits[:limit]
                new_insts.append(inst)
            bb.instructions = new_insts
    return n


def _get_nc():
    if "nc" not in _BUILT:
        _BUILT["nc"] = _build_nc()
    return _BUILT["nc"]


def _prep_core_inputs(x_b, W_q, b_q, W_k, b_k, W_v, b_v, W_o, g):
    """Inputs for one core: batch slice x_b [S, D], head group g (0/1)."""
    import ml_dtypes
    bf16 = ml_dtypes.bfloat16
    hs = slice(g * HPC, (g + 1) * HPC)

    # xT: [p][c][k][sc] = x_b[512c+sc, 128k+p]
    xT = np.ascontiguousarray(
        x_b.reshape(SB, 512, KT, 128).transpose(3, 0, 2, 1)
        .reshape(128, SB * KT * 512)).astype(bf16)

    def arrange_qk(wfull):  # [D, 512] -> [128, NPAIR*KT*128] pair-major
        return np.ascontiguousarray(
            wfull.reshape(KT, 128, NPAIR, 128).transpose(1, 2, 0, 3)
            .reshape(128, NPAIR * KT * 128))

    wq = arrange_qk(W_q[hs].transpose(1, 0, 2).reshape(D, 512)).astype(bf16)
    wk = arrange_qk(W_k[hs].transpose(1, 0, 2).reshape(D, 512)).astype(bf16)
    wv = np.ascontiguousarray(
        W_v[hs].transpose(1, 0, 2).reshape(D, 512)
        .reshape(KT, 128, 512).transpose(1, 0, 2)
        .reshape(128, KT * 512)).astype(bf16)
    wo_t = np.ascontiguousarray(W_o[:, g * 512:(g + 1) * 512].T)  # [512, D]
    wo = np.ascontiguousarray(
        wo_t.reshape(NPAIR, 128, D).transpose(1, 0, 2).reshape(128, NPAIR * D)
    ).astype(bf16)
    bq = np.ascontiguousarray(
        b_q[hs].reshape(NPAIR, 128).T).astype(np.float32)          # [128, 4]
    bk = np.ascontiguousarray(
        b_k[hs].reshape(NPAIR, 128).T).astype(np.float32)
    bv = np.ascontiguousarray(np.broadcast_to(
        b_v[hs].reshape(1, 512), (128, 512))).astype(np.float32)   # [128, 512]

    p = np.arange(128)[:, None]
    cc = np.arange(512)[None, :]
    mask = (cc >= p).astype(bf16)                                  # [128, 512]

    return {"xT": xT, "wq": wq, "wk": wk, "wv": wv, "wo": wo,
            "bq": bq, "bk": bk, "bv": bv, "mask": mask}


def _install_axon_ntff_hook():
    """Register the axon NTFF profiling hook if the environment allows.

    The agent image lacks ``antenv.axon_hooks``; synthesize it and wire the
    ctypes-based profiler from trn_agent_boot so BASS_TRACE=1 yields NTFFs.
    Degrades silently — without it run_bass_kernel_spmd(trace=False) works.
    """
    import sys
    import types
    try:
        import antenv
        if "antenv.axon_hooks" not in sys.modules:
            mod = types.ModuleType("antenv.axon_hooks")
            holder = [None]
            mod.set_axon_ntff_profile_hook = lambda h: holder.__setitem__(0, h)
            mod.get_axon_ntff_profile_hook = lambda: holder[0]
            sys.modules["antenv.axon_hooks"] = mod
            antenv.axon_hooks = mod
        mod = sys.modules["antenv.axon_hooks"]
        if mod.get_axon_ntff_profile_hook() is None:
            from trn_agent_boot.trn_boot import _ntff_profile_via_ctypes
            hook = _ntff_profile_via_ctypes("/opt/axon/libaxon_pjrt.so")
            mod.set_axon_ntff_profile_hook(hook)
        import concourse.bass_utils as bu
        bu.upload_artifacts = lambda d: d  # no S3 in this container
    except Exception:
        pass


def kernel(inputs, W_q, b_q, W_k, b_k, W_v, b_v, W_o, b_o):
    global LAST_RESULTS
    from concourse.bass_utils import run_bass_kernel_spmd
    _install_axon_ntff_hook()

    inputs = np.asarray(inputs, dtype=np.float32)
    args = [np.asarray(a, dtype=np.float32)
            for a in (W_q, b_q, W_k, b_k, W_v, b_v, W_o, b_o)]
    W_q, b_q, W_k, b_k, W_v, b_v, W_o, b_o = args

    nc = _get_nc()
    in_maps = []
    for c in range(NCORES):
        bi, g = c // 2, c % 2
        in_maps.append(_prep_core_inputs(
            inputs[bi], W_q, b_q, W_k, b_k, W_v, b_v, W_o, g))

    res = run_bass_kernel_spmd(nc, in_maps, list(range(NCORES)))
    LAST_RESULTS = res

    out = np.empty((B, S, D), dtype=np.float32)
    for bi in range(B):
        out[bi] = (res.results[2 * bi]["out"] + res.results[2 * bi + 1]["out"]
                   + b_o[None, :])
    return out



# revision 4
# speedup vs baseline: 1.0202x; 1.0202x over previous
"""Causal multi-head attention TRN2 kernel (8 NeuronCores).

Problem: B=4, S=2048, D=1024, H=16 heads, head_dim=64 (fp32 reference).

Sharding: data-parallel over batch (4) x tensor-parallel over head-groups (2).
Core c handles batch c//2 with heads (c%2)*8 .. (c%2)*8+8 and produces a
partial [S, D] output (its head-group's contribution to the O-projection,
without b_o) in bf16. Host sums the two partials per batch and adds b_o.

Block-major schedule: attention q-blocks are processed in order b=0..3 with
all 4 head-pairs per block, so the O-projection s-tiles, softmax-denominator
normalization chains and output DMAs of block b all retire during block b+1
instead of piling into the kernel tail. PE warm-up matmuls run during the
initial DMA wait so the HAM clock gate opens before real work arrives.
"""

import math

import numpy as np

B, S, D, H = 4, 2048, 1024, 16
HD = D // H        # 64
NCORES = 8
HPC = H // 2       # heads per core: 8
NPAIR = HPC // 2   # head pairs per core: 4
KT = D // 128      # contraction tiles: 8
ST = S // 128      # seq tiles of 128: 16
SB = S // 512      # seq blocks of 512: 4

_BUILT = {}
LAST_RESULTS = None  # BassKernelResults of the most recent run (for test.py)


def _build_nc():
    import concourse.bass as bass
    import concourse.mybir as mybir
    from concourse import tile

    f32 = mybir.dt.float32
    bf16 = mybir.dt.bfloat16
    AF = mybir.ActivationFunctionType
    OP = mybir.AluOpType

    nc = bass.Bass("TRN2", target_bir_lowering=False, debug=False,
                   num_devices=NCORES)

    # DRAM layouts are pre-arranged on the host to match SBUF tiles exactly.
    # xT: col = c*4096 + k*512 + sc  (block-major so block 0 lands first)
    # wq/wk: col = j*1024 + k*128 + e  (pair-major so pair 0 lands first)
    xT_d = nc.dram_tensor("xT", [128, SB * KT * 512], bf16,
                          kind="ExternalInput").ap()
    wq_d = nc.dram_tensor("wq", [128, NPAIR * KT * 128], bf16,
                          kind="ExternalInput").ap()
    wk_d = nc.dram_tensor("wk", [128, NPAIR * KT * 128], bf16,
                          kind="ExternalInput").ap()
    wv_d = nc.dram_tensor("wv", [128, KT * 512], bf16,
                          kind="ExternalInput").ap()
    wo_d = nc.dram_tensor("wo", [128, NPAIR * D], bf16,
                          kind="ExternalInput").ap()
    bq_d = nc.dram_tensor("bq", [128, NPAIR], f32, kind="ExternalInput").ap()
    bk_d = nc.dram_tensor("bk", [128, NPAIR], f32, kind="ExternalInput").ap()
    bv_d = nc.dram_tensor("bv", [128, 512], f32, kind="ExternalInput").ap()
    mask_d = nc.dram_tensor("mask", [128, 512], bf16,
                            kind="ExternalInput").ap()
    out_d = nc.dram_tensor("out", [S, D], bf16, kind="ExternalOutput").ap()

    with tile.TileContext(nc) as tc:
        with tc.tile_pool(name="persist", bufs=1) as pp, \
             tc.tile_pool(name="pacc", bufs=2, space="PSUM") as pacc, \
             tc.tile_pool(name="pss", bufs=2, space="PSUM") as pssp, \
             tc.tile_pool(name="pwv", bufs=1, space="PSUM") as pwvp, \
             tc.tile_pool(name="attn", bufs=6) as attnp, \
             tc.tile_pool(name="norm", bufs=3) as normp, \
             tc.tile_pool(name="ost", bufs=3) as ostp, \
             tc.tile_pool(name="scr", bufs=4, space="DRAM") as scrp:

            xt_all = pp.tile([128, SB * KT * 512], bf16, tag="xt", name="xt")
            wq_all = pp.tile([128, NPAIR * KT * 128], bf16, tag="wq", name="wq")
            wk_all = pp.tile([128, NPAIR * KT * 128], bf16, tag="wk", name="wk")
            wv_all = pp.tile([128, KT * 512], bf16, tag="wv", name="wv")
            wo_all = pp.tile([128, NPAIR * D], bf16, tag="wo", name="wo")
            bq = pp.tile([128, NPAIR], f32, tag="bq", name="bq")
            bk = pp.tile([128, NPAIR], f32, tag="bk", name="bk")
            bv = pp.tile([128, 512], f32, tag="bv", name="bv")
            mask = pp.tile([128, 512], bf16, tag="mask", name="mask")
            qT = [pp.tile([128, S], bf16, tag=f"qT{j}", name=f"qT{j}")
                  for j in range(NPAIR)]
            kTt = [pp.tile([128, S], bf16, tag=f"kT{j}", name=f"kT{j}")
                   for j in range(NPAIR)]
            v_all = [pp.tile([128, 8 * 65], bf16, tag=f"v{t}", name=f"v{t}")
                     for t in range(ST)]
            wvT = [pp.tile([128, S], bf16, tag=f"wvT{j}", name=f"wvT{j}")
                   for j in range(NPAIR)]
            junk = pp.tile([128, 512], bf16, tag="junk", name="junk")

            # ---- input DMAs ----
            # Three issue streams (scalar/sync/gpsimd) round-robin on fabric
            # bandwidth; order each stream by first-use time.  Block-major
            # needs all four pairs' W_q/W_k for block 0, so weights stream on
            # the scalar queue while x block 0 splits across sync+gpsimd.
            nc.scalar.dma_start(wq_all[:, 0:1024], wq_d[:, 0:1024])
            nc.scalar.dma_start(wk_all[:, 0:1024], wk_d[:, 0:1024])
            nc.scalar.dma_start(bq, bq_d[:, :])
            nc.scalar.dma_start(bk, bk_d[:, :])
            nc.gpsimd.dma_start(mask, mask_d[:, :])
            for i in range(4):           # x block 0: 8 fine chunks
                cs = slice(i * 512, (i + 1) * 512)
                nc.sync.dma_start(xt_all[:, cs], xT_d[:, cs])
            for i in range(4, 8):
                cs = slice(i * 512, (i + 1) * 512)
                nc.gpsimd.dma_start(xt_all[:, cs], xT_d[:, cs])
            nc.sync.dma_start(bv, bv_d[:, :])
            nc.sync.dma_start(wv_all[:, 0:2048], wv_d[:, 0:2048])
            nc.gpsimd.dma_start(wv_all[:, 2048:4096], wv_d[:, 2048:4096])
            # remaining W_q/W_k pairs 1-3 (needed as block-0 fillers)
            nc.scalar.dma_start(wq_all[:, 1024:2560], wq_d[:, 1024:2560])
            nc.scalar.dma_start(wk_all[:, 1024:2560], wk_d[:, 1024:2560])
            nc.scalar.dma_start(wq_all[:, 2560:4096], wq_d[:, 2560:4096])
            nc.scalar.dma_start(wk_all[:, 2560:4096], wk_d[:, 2560:4096])
            # x blocks 1-3
            for c in range(1, SB):
                cs0 = slice(c * 4096, c * 4096 + 2048)
                cs1 = slice(c * 4096 + 2048, (c + 1) * 4096)
                nc.sync.dma_start(xt_all[:, cs0], xT_d[:, cs0])
                nc.gpsimd.dma_start(xt_all[:, cs1], xT_d[:, cs1])
            nc.scalar.dma_start(wo_all, wo_d[:, :])

            # ---- PE warm-up ----
            # ~3.4us of junk matmuls (no DMA deps) flips the HAM clock gate
            # to K=8/8 while the first x/weight chunks stream in, so real
            # chains start at 2.4 GHz.  Output bank is never read.
            nc.gpsimd.memset(junk[:, :], 0.0)
            jps = pacc.tile([128, 512], f32, tag="acc", name="acc")
            for i in range(8):
                nc.tensor.matmul(jps[:, :], lhsT=junk[:, 0:128],
                                 rhs=junk[:, :], start=(i == 0),
                                 stop=(i == 7))
            # prewarm the ACT exp table (first activation triggers a ~2.7us
            # ACT_TABLE_LOAD; don't pay it on the first real softmax tile)
            warm = pp.tile([128, 1], f32, tag="warm", name="warm")
            nc.gpsimd.memset(warm[:, :], 0.0)
            nc.scalar.activation(warm[:, :], warm[:, :], AF.Exp, scale=1.0)

            def xt(k, c):
                return xt_all[:, c * 4096 + k * 512:c * 4096 + (k + 1) * 512]

            wvk = [wv_all[:, 512 * k:512 * (k + 1)] for k in range(KT)]
            wo = [wo_all[:, D * j:D * (j + 1)] for j in range(NPAIR)]

            def emit_qk_chain(j, c, which):
                cs = slice(512 * c, 512 * c + 512)
                ps = pacc.tile([128, 512], f32, tag="acc", name="acc")
                w_all = wq_all if which == 'q' else wk_all
                dst = qT[j] if which == 'q' else kTt[j]
                bias = bq if which == 'q' else bk
                for k in range(KT):
                    nc.tensor.matmul(
                        ps[:, :],
                        lhsT=w_all[:, j * 1024 + k * 128:j * 1024 + (k + 1) * 128],
                        rhs=xt(k, c),
                        start=(k == 0), stop=(k == KT - 1))
                nc.vector.tensor_scalar_add(dst[:, cs], ps[:, :],
                                            bias[:, j:j + 1])

            def emit_v(t):
                ps = pacc.tile([128, 512], f32, tag="acc", name="acc")
                c, i = t // 4, t % 4
                for k in range(KT):
                    nc.tensor.matmul(
                        ps[:, :],
                        lhsT=xt(k, c)[:, 128 * i:128 * i + 128],
                        rhs=wvk[k],
                        start=(k == 0), stop=(k == KT - 1))
                nc.gpsimd.memset(
                    v_all[t].rearrange("p (h e) -> p h e", e=65)[:, :, 64:65],
                    1.0)
                nc.vector.tensor_tensor(
                    v_all[t].rearrange("p (h e) -> p h e", e=65)[:, :, 0:64],
                    ps.rearrange("p (h e) -> p h e", e=64),
                    bv.rearrange("p (h e) -> p h e", e=64),
                    op=OP.add)

            finishers = []

            def emit_attn_block(j, b, fillers, fill_every=1):
                nt = 4 * b + 4
                wvA = pwvp.tile([65, 512], f32, tag="wvA", name="wvA")
                wvB = pwvp.tile([65, 512], f32, tag="wvB", name="wvB")
                live = {}

                def scores(t):
                    off = max(0, 128 * t - 512 * b)
                    qs = slice(512 * b + off, 512 * b + 512)
                    ts = slice(128 * t, 128 * t + 128)
                    ps = pssp.tile([128, 1024], f32, tag="pss", name="pss")
                    nc.tensor.matmul(
                        ps[:, off:512], lhsT=kTt[j][0:64, ts],
                        rhs=qT[j][0:64, qs], start=True, stop=True,
                        tile_position=(0, 0))
                    nc.tensor.matmul(
                        ps[:, 512 + off:1024], lhsT=kTt[j][64:128, ts],
                        rhs=qT[j][64:128, qs], start=True, stop=True,
                        tile_position=(64, 0))
                    live[t] = (ps, attnp.tile([128, 1024], bf16, tag="at",
                                              name="at"), off)

                def expmask(t):
                    ps, at, off = live[t]
                    if off:
                        nc.scalar.activation(
                            at.rearrange("p (h w) -> p h w", h=2)[:, :, off:512],
                            ps.rearrange("p (h w) -> p h w", h=2)[:, :, off:512],
                            AF.Exp, scale=0.125)
                    else:
                        nc.scalar.activation(at[:, :], ps[:, :], AF.Exp,
                                             scale=0.125)
                    if t >= 4 * b:
                        w = 512 - off
                        nc.vector.tensor_tensor(
                            at[:, off:512], at[:, off:512], mask[:, 0:w],
                            op=OP.mult)
                        nc.vector.tensor_tensor(
                            at[:, 512 + off:1024], at[:, 512 + off:1024],
                            mask[:, 0:w], op=OP.mult)

                def wv(t):
                    ps, at, off = live.pop(t)
                    nc.tensor.matmul(
                        wvA[:, off:512],
                        lhsT=v_all[t][:, 130 * j:130 * j + 65],
                        rhs=at[:, off:512],
                        start=(t == 0), stop=(t == nt - 1))
                    nc.tensor.matmul(
                        wvB[:, off:512],
                        lhsT=v_all[t][:, 130 * j + 65:130 * j + 130],
                        rhs=at[:, 512 + off:1024],
                        start=(t == 0), stop=(t == nt - 1))

                scores(0)
                expmask(0)
                for t in range(1, nt):
                    scores(t)
                    wv(t - 1)
                    expmask(t)
                    if t == 2 and finishers:
                        finishers.pop(0)()
                    if t % fill_every == 0 and fillers:
                        fillers.pop(0)()
                wv(nt - 1)

                # drain wv PSUM immediately (frees the banks for the next
                # pair) — wvA on the vector queue, wvB on gpsimd so the two
                # copies land in parallel before the next pair's first wv;
                # then kick off the denominator redistribute.  The DVE
                # reciprocal is deferred into the NEXT pair's attention so
                # it never stalls the in-order DVE queue ahead of that
                # pair's masks.
                bs = slice(512 * b, 512 * b + 512)
                stg = normp.tile([65, 1024], f32, tag="stg", name="stg")
                nc.vector.tensor_copy(stg[:, 0:512], wvA[:, :])
                nc.vector.tensor_copy(stg[:, 512:1024], wvB[:, :])
                sumsq = normp.tile([128, 8], f32, tag="sumsq", name="sumsq")
                nc.sync.dma_start(sumsq[:, :], stg[64:65, :])

                def finish(j=j, b=b, stg=stg, sumsq=sumsq, bs=bs):
                    rq = normp.tile([128, 8], f32, tag="rq", name="rq")
                    nc.vector.reciprocal(rq[:, :], sumsq[:, :])
                    scr = scrp.tile([1, 1024], f32, tag="scr", name="scr")
                    nc.sync.dma_start(scr[:, :], rq[:, :])
                    rsb = normp.tile([64, 1024], f32, tag="rsb", name="rsb")
                    nc.sync.dma_start(rsb[:, :],
                                      scr[0:1, :].broadcast_to((64, 1024)))
                    nc.gpsimd.tensor_tensor(wvT[j][0:64, bs],
                                            stg[0:64, 0:512],
                                            rsb[:, 0:512], op=OP.mult)
                    nc.gpsimd.tensor_tensor(wvT[j][64:128, bs],
                                            stg[0:64, 512:1024],
                                            rsb[:, 512:1024], op=OP.mult)
                finishers.append(finish)

            def emit_oproj_stile(s):
                ss = slice(128 * s, 128 * s + 128)
                ost = ostp.tile([128, 1024], bf16, tag="ost", name="ost")
                for n in range(2):
                    ns = slice(512 * n, 512 * n + 512)
                    ps = pacc.tile([128, 512], f32, tag="acc", name="acc")
                    for j in range(NPAIR):
                        nc.tensor.matmul(
                            ps[:, :], lhsT=wvT[j][:, ss], rhs=wo[j][:, ns],
                            start=(j == 0), stop=(j == NPAIR - 1))
                    nc.vector.tensor_copy(ost[:, ns], ps[:, :])
                nc.scalar.dma_start(out_d[ss, :], ost[:, :])

            # ---- emission schedule (block-major) ----
            # chains (j, c) are emitted during block c-1; v tiles for block b
            # land as fillers just before/inside the first pair of block b;
            # oproj s-tiles of block b are deferred as late as dependencies
            # allow so the late blocks (largest ACT share) keep the PE fed.
            ch = lambda j, c, w: (lambda: emit_qk_chain(j, c, w))
            vt = lambda t: (lambda: emit_v(t))
            os_ = lambda s: (lambda: emit_oproj_stile(s))

            emit_qk_chain(0, 0, 'q')
            emit_qk_chain(0, 0, 'k')
            for t in range(4):
                emit_v(t)

            # block 0 (nt=4: 3 filler slots per pair)
            emit_attn_block(0, 0, [ch(1, 0, 'q'), ch(1, 0, 'k')])
            emit_attn_block(1, 0, [ch(2, 0, 'q'), ch(2, 0, 'k'),
                                   ch(0, 1, 'q')])
            emit_attn_block(2, 0, [ch(3, 0, 'q'), ch(3, 0, 'k'),
                                   ch(0, 1, 'k')])
            emit_attn_block(3, 0, [vt(4), vt(5), ch(1, 1, 'q')])
            emit_v(6)
            emit_v(7)

            # block 1 (nt=8: 7 slots per pair)
            emit_attn_block(0, 1, [ch(1, 1, 'k'), ch(2, 1, 'q'),
                                   ch(2, 1, 'k')])
            emit_attn_block(1, 1, [ch(3, 1, 'q'), ch(3, 1, 'k'), os_(0)])
            emit_attn_block(2, 1, [ch(0, 2, 'q'), ch(0, 2, 'k'), vt(8),
                                   os_(1)])
            emit_attn_block(3, 1, [ch(1, 2, 'q'), ch(1, 2, 'k'), vt(9),
                                   vt(10)])
            emit_v(11)

            # block 2 (nt=12: 11 slots per pair)
            emit_attn_block(0, 2, [ch(2, 2, 'q'), ch(2, 2, 'k'), os_(2)])
            emit_attn_block(1, 2, [ch(3, 2, 'q'), ch(3, 2, 'k'), os_(3)])
            emit_attn_block(2, 2, [ch(0, 3, 'q'), ch(0, 3, 'k'), vt(12)])
            emit_attn_block(3, 2, [ch(1, 3, 'q'), ch(1, 3, 'k'), vt(13),
                                   vt(14)])
            emit_v(15)

            # block 3 (nt=16: 15 slots per pair)
            emit_attn_block(0, 3, [ch(2, 3, 'q'), ch(2, 3, 'k'), os_(4),
                                   os_(5)])
            emit_attn_block(1, 3, [ch(3, 3, 'q'), ch(3, 3, 'k'), os_(6),
                                   os_(7)])
            emit_attn_block(2, 3, [os_(8), os_(9), os_(10)])
            emit_attn_block(3, 3, [os_(11)])

            # ---- tail ----
            # finisher (3,3) immediately; overlap its DMA-chain latency with
            # the j=0..2 partial accumulation of oproj s-tile 12.
            while finishers:
                finishers.pop(0)()
            s = 12
            ss = slice(128 * s, 128 * s + 128)
            tail_ps = []
            for n in range(2):
                ps = pacc.tile([128, 512], f32, tag="acc", name="acc")
                for j in range(3):
                    nc.tensor.matmul(
                        ps[:, :], lhsT=wvT[j][:, ss],
                        rhs=wo[j][:, 512 * n:512 * n + 512],
                        start=(j == 0), stop=False)
                tail_ps.append(ps)
            ost = ostp.tile([128, 1024], bf16, tag="ost", name="ost")
            for n in range(2):
                nc.tensor.matmul(
                    tail_ps[n][:, :], lhsT=wvT[3][:, ss],
                    rhs=wo[3][:, 512 * n:512 * n + 512],
                    start=False, stop=True)
                nc.vector.tensor_copy(ost[:, 512 * n:512 * n + 512],
                                      tail_ps[n][:, :])
            nc.scalar.dma_start(out_d[ss, :], ost[:, :])
            emit_oproj_stile(13)
            emit_oproj_stile(14)
            emit_oproj_stile(15)

    _split_excess_waits(nc, limit=1)
    return nc


def _split_excess_waits(nc, limit=1):
    """This container's walrus encodes at most one sem wait per instruction;
    move excess waits onto standalone EventSemaphore ops just before each
    over-limit instruction (same engine stream, so semantics preserved)."""
    import concourse.mybir as mybir
    n = 0
    for fn in nc.m.functions:
        for bb in fn.blocks:
            new_insts = []
            for inst in bb.instructions:
                si = inst.sync_info
                if si is not None and si.on_wait and len(si.on_wait) > limit:
                    waits = list(si.on_wait)
                    for i, w in enumerate(waits[limit:]):
                        wi = mybir.InstEventSemaphore(
                            name=f"{inst.name}-wsplit{i}", ins=[], outs=[],
                            sync_info=mybir.SyncInfo(on_wait=[w], on_update=[]))
                        wi.engine = inst.engine
                        nc.register_instruction(wi)
                        new_insts.append(wi)
                        n += 1
                    si.on_wait = waits[:limit]
                new_insts.append(inst)
            bb.instructions = new_insts
    return n


def _get_nc():
    if "nc" not in _BUILT:
        _BUILT["nc"] = _build_nc()
    return _BUILT["nc"]


def _prep_core_inputs(x_b, W_q, b_q, W_k, b_k, W_v, b_v, W_o, g):
    """Inputs for one core: batch slice x_b [S, D], head group g (0/1)."""
    import ml_dtypes
    bf16 = ml_dtypes.bfloat16
    hs = slice(g * HPC, (g + 1) * HPC)

    # xT: [p][c][k][sc] = x_b[512c+sc, 128k+p]
    xT = np.ascontiguousarray(
        x_b.reshape(SB, 512, KT, 128).transpose(3, 0, 2, 1)
        .reshape(128, SB * KT * 512)).astype(bf16)

    def arrange_qk(wfull):  # [D, 512] -> [128, NPAIR*KT*128] pair-major
        return np.ascontiguousarray(
            wfull.reshape(KT, 128, NPAIR, 128).transpose(1, 2, 0, 3)
            .reshape(128, NPAIR * KT * 128))

    wq = arrange_qk(W_q[hs].transpose(1, 0, 2).reshape(D, 512)).astype(bf16)
    wk = arrange_qk(W_k[hs].transpose(1, 0, 2).reshape(D, 512)).astype(bf16)
    wv = np.ascontiguousarray(
        W_v[hs].transpose(1, 0, 2).reshape(D, 512)
        .reshape(KT, 128, 512).transpose(1, 0, 2)
        .reshape(128, KT * 512)).astype(bf16)
    wo_t = np.ascontiguousarray(W_o[:, g * 512:(g + 1) * 512].T)  # [512, D]
    wo = np.ascontiguousarray(
        wo_t.reshape(NPAIR, 128, D).transpose(1, 0, 2).reshape(128, NPAIR * D)
    ).astype(bf16)
    bq = np.ascontiguousarray(
        b_q[hs].reshape(NPAIR, 128).T).astype(np.float32)          # [128, 4]
    bk = np.ascontiguousarray(
        b_k[hs].reshape(NPAIR, 128).T).astype(np.float32)
    bv = np.ascontiguousarray(np.broadcast_to(
        b_v[hs].reshape(1, 512), (128, 512))).astype(np.float32)   # [128, 512]

    p = np.arange(128)[:, None]
    cc = np.arange(512)[None, :]
    mask = (cc >= p).astype(bf16)                                  # [128, 512]

    return {"xT": xT, "wq": wq, "wk": wk, "wv": wv, "wo": wo,
            "bq": bq, "bk": bk, "bv": bv, "mask": mask}


def _install_axon_ntff_hook():
    """Register the axon NTFF profiling hook if the environment allows.

    The agent image lacks ``antenv.axon_hooks``; synthesize it and wire the
    ctypes-based profiler from trn_agent_boot so BASS_TRACE=1 yields NTFFs.
    Degrades silently — without it run_bass_kernel_spmd(trace=False) works.
    """
    import sys
    import types
    try:
        import antenv
        if "antenv.axon_hooks" not in sys.modules:
            mod = types.ModuleType("antenv.axon_hooks")
            holder = [None]
            mod.set_axon_ntff_profile_hook = lambda h: holder.__setitem__(0, h)
            mod.get_axon_ntff_profile_hook = lambda: holder[0]
            sys.modules["antenv.axon_hooks"] = mod
            antenv.axon_hooks = mod
        mod = sys.modules["antenv.axon_hooks"]
        if mod.get_axon_ntff_profile_hook() is None:
            from trn_agent_boot.trn_boot import _ntff_profile_via_ctypes
            hook = _ntff_profile_via_ctypes("/opt/axon/libaxon_pjrt.so")
            mod.set_axon_ntff_profile_hook(hook)
        import concourse.bass_utils as bu
        bu.upload_artifacts = lambda d: d  # no S3 in this container
    except Exception:
        pass


def kernel(inputs, W_q, b_q, W_k, b_k, W_v, b_v, W_o, b_o):
    global LAST_RESULTS
    from concourse.bass_utils import run_bass_kernel_spmd
    _install_axon_ntff_hook()

    inputs = np.asarray(inputs, dtype=np.float32)
    args = [np.asarray(a, dtype=np.float32)
            for a in (W_q, b_q, W_k, b_k, W_v, b_v, W_o, b_o)]
    W_q, b_q, W_k, b_k, W_v, b_v, W_o, b_o = args

    nc = _get_nc()
    in_maps = []
    for c in range(NCORES):
        bi, g = c // 2, c % 2
        in_maps.append(_prep_core_inputs(
            inputs[bi], W_q, b_q, W_k, b_k, W_v, b_v, W_o, g))

    res = run_bass_kernel_spmd(nc, in_maps, list(range(NCORES)))
    LAST_RESULTS = res

    out = np.empty((B, S, D), dtype=np.float32)
    for bi in range(B):
        out[bi] = (res.results[2 * bi]["out"].astype(np.float32)
                   + res.results[2 * bi + 1]["out"].astype(np.float32)
                   + b_o[None, :])
    return out


# revision 9
# speedup vs baseline: 1.0607x; 1.0398x over previous
"""Causal multi-head attention TRN2 kernel (8 NeuronCores).

Problem: B=4, S=2048, D=1024, H=16 heads, head_dim=64 (fp32 reference).

Sharding: data-parallel over batch (4) x tensor-parallel over head-groups (2).
Core c handles batch c//2 with heads (c%2)*8 .. (c%2)*8+8 and produces a
partial [S, D] output (its head-group's contribution to the O-projection,
without b_o) in bf16. Host sums the two partials per batch and adds b_o.

Block-major schedule: attention q-blocks are processed in order b=0..3 with
all 4 head-pairs per block, so the O-projection s-tiles, softmax-denominator
normalization chains and output DMAs of block b all retire during block b+1
instead of piling into the kernel tail. PE warm-up matmuls run during the
initial DMA wait so the HAM clock gate opens before real work arrives.
"""

import math

import numpy as np

B, S, D, H = 4, 2048, 1024, 16
HD = D // H        # 64
NCORES = 8
HPC = H // 2       # heads per core: 8
NPAIR = HPC // 2   # head pairs per core: 4
KT = D // 128      # contraction tiles: 8
ST = S // 128      # seq tiles of 128: 16
SB = S // 512      # seq blocks of 512: 4

_BUILT = {}
LAST_RESULTS = None  # BassKernelResults of the most recent run (for test.py)


def _build_nc():
    import concourse.bass as bass
    import concourse.mybir as mybir
    from concourse import tile

    f32 = mybir.dt.float32
    bf16 = mybir.dt.bfloat16
    AF = mybir.ActivationFunctionType
    OP = mybir.AluOpType

    nc = bass.Bass("TRN2", target_bir_lowering=False, debug=False,
                   num_devices=NCORES)

    # DRAM layouts are pre-arranged on the host to match SBUF tiles exactly.
    # xT: col = c*4096 + k*512 + sc  (block-major so block 0 lands first)
    # wq/wk: col = j*1024 + k*128 + e  (pair-major so pair 0 lands first)
    xT_d = nc.dram_tensor("xT", [128, SB * KT * 512], bf16,
                          kind="ExternalInput").ap()
    wq_d = nc.dram_tensor("wq", [128, NPAIR * KT * 128], bf16,
                          kind="ExternalInput").ap()
    wk_d = nc.dram_tensor("wk", [128, NPAIR * KT * 128], bf16,
                          kind="ExternalInput").ap()
    wv_d = nc.dram_tensor("wv", [128, KT * 512], bf16,
                          kind="ExternalInput").ap()
    wo_d = nc.dram_tensor("wo", [128, NPAIR * D], bf16,
                          kind="ExternalInput").ap()
    bq_d = nc.dram_tensor("bq", [128, NPAIR], f32, kind="ExternalInput").ap()
    bk_d = nc.dram_tensor("bk", [128, NPAIR], f32, kind="ExternalInput").ap()
    bv_d = nc.dram_tensor("bv", [128, 512], f32, kind="ExternalInput").ap()
    mask_d = nc.dram_tensor("mask", [128, 512], bf16,
                            kind="ExternalInput").ap()
    out_d = nc.dram_tensor("out", [S, D], bf16, kind="ExternalOutput").ap()

    with tile.TileContext(nc) as tc:
        with tc.tile_pool(name="persist", bufs=1) as pp, \
             tc.tile_pool(name="pacc", bufs=2, space="PSUM") as pacc, \
             tc.tile_pool(name="pss", bufs=2, space="PSUM") as pssp, \
             tc.tile_pool(name="pwv", bufs=1, space="PSUM") as pwvp, \
             tc.tile_pool(name="attn", bufs=6) as attnp, \
             tc.tile_pool(name="norm", bufs=3) as normp, \
             tc.tile_pool(name="ost", bufs=3) as ostp, \
             tc.tile_pool(name="scr", bufs=4, space="DRAM") as scrp:

            xt_all = pp.tile([128, SB * KT * 512], bf16, tag="xt", name="xt")
            wq_all = pp.tile([128, NPAIR * KT * 128], bf16, tag="wq", name="wq")
            wk_all = pp.tile([128, NPAIR * KT * 128], bf16, tag="wk", name="wk")
            wv_all = pp.tile([128, KT * 512], bf16, tag="wv", name="wv")
            wo_all = pp.tile([128, NPAIR * D], bf16, tag="wo", name="wo")
            bq = pp.tile([128, NPAIR], f32, tag="bq", name="bq")
            bk = pp.tile([128, NPAIR], f32, tag="bk", name="bk")
            bv = pp.tile([128, 512], f32, tag="bv", name="bv")
            mask = pp.tile([128, 512], bf16, tag="mask", name="mask")
            qT = [pp.tile([128, S], bf16, tag=f"qT{j}", name=f"qT{j}")
                  for j in range(NPAIR)]
            kTt = [pp.tile([128, S], bf16, tag=f"kT{j}", name=f"kT{j}")
                   for j in range(NPAIR)]
            v_all = [pp.tile([128, 8 * 65], bf16, tag=f"v{t}", name=f"v{t}")
                     for t in range(ST)]
            wvT = [pp.tile([128, S], bf16, tag=f"wvT{j}", name=f"wvT{j}")
                   for j in range(NPAIR)]
            junk = pp.tile([128, 512], bf16, tag="junk", name="junk")

            ones1 = pp.tile([65, 128], f32, tag="ones1", name="ones1")

            # ---- input DMAs ----
            # Three issue streams (scalar/sync/gpsimd) round-robin on fabric
            # bandwidth; order each stream by first-use time.  Each trigger
            # instruction costs ~0.65us on its issuing engine, so chunks are
            # kept big.  Block-major needs all four pairs' W_q/W_k for block
            # 0, so weights stream on scalar while x block 0 splits across
            # sync+gpsimd.
            nc.scalar.dma_start(wq_all[:, 0:1024], wq_d[:, 0:1024])
            nc.scalar.dma_start(wk_all[:, 0:1024], wk_d[:, 0:1024])
            nc.scalar.dma_start(bq, bq_d[:, :])
            nc.scalar.dma_start(bk, bk_d[:, :])
            nc.sync.dma_start(mask, mask_d[:, :])
            nc.sync.dma_start(xt_all[:, 0:1024], xT_d[:, 0:1024])
            nc.sync.dma_start(xt_all[:, 1024:2048], xT_d[:, 1024:2048])
            nc.gpsimd.dma_start(xt_all[:, 2048:3072], xT_d[:, 2048:3072])
            nc.gpsimd.dma_start(xt_all[:, 3072:4096], xT_d[:, 3072:4096])
            nc.scalar.dma_start(wv_all[:, 3072:4096], wv_d[:, 3072:4096])
            nc.sync.dma_start(wv_all[:, 0:1536], wv_d[:, 0:1536])
            nc.gpsimd.dma_start(wv_all[:, 1536:3072], wv_d[:, 1536:3072])
            nc.sync.dma_start(bv, bv_d[:, :])
            # remaining W_q/W_k pairs 1-3 (needed as block-0 fillers)
            nc.scalar.dma_start(wq_all[:, 1024:2560], wq_d[:, 1024:2560])
            nc.scalar.dma_start(wk_all[:, 1024:2560], wk_d[:, 1024:2560])
            nc.scalar.dma_start(wq_all[:, 2560:4096], wq_d[:, 2560:4096])
            nc.scalar.dma_start(wk_all[:, 2560:4096], wk_d[:, 2560:4096])
            # x blocks 1-3
            for c in range(1, SB):
                cs0 = slice(c * 4096, c * 4096 + 2048)
                cs1 = slice(c * 4096 + 2048, (c + 1) * 4096)
                nc.sync.dma_start(xt_all[:, cs0], xT_d[:, cs0])
                nc.gpsimd.dma_start(xt_all[:, cs1], xT_d[:, cs1])
            nc.scalar.dma_start(wo_all, wo_d[:, :])

            # ---- PE warm-up ----
            # ~6us of junk matmuls (no DMA deps; memset on the otherwise-idle
            # vector queue so it issues right after the preamble) flips the
            # HAM clock gate to K=8/8 and keeps the PE busy until the first
            # x/weight chunks land (~12us: ~7.5us fixed preamble before any
            # DMA issues + ~4.5us of transfer).  Output bank is never read.
            nc.vector.memset(junk[:, :], 0.0)
            nc.vector.memset(ones1[:, :], 1.0)
            jps = pacc.tile([128, 512], f32, tag="acc", name="acc")
            for i in range(14):
                nc.tensor.matmul(jps[:, :], lhsT=junk[:, 0:128],
                                 rhs=junk[:, :], start=(i == 0),
                                 stop=(i == 13))
            # prewarm the ACT exp table (first activation triggers a ~2.7us
            # ACT_TABLE_LOAD; don't pay it on the first real softmax tile)
            warm = pp.tile([128, 1], f32, tag="warm", name="warm")
            nc.vector.memset(warm[:, :], 0.0)
            nc.scalar.activation(warm[:, :], warm[:, :], AF.Exp, scale=1.0)

            def xt(k, c):
                return xt_all[:, c * 4096 + k * 512:c * 4096 + (k + 1) * 512]

            wvk = [wv_all[:, 512 * k:512 * (k + 1)] for k in range(KT)]
            wo = [wo_all[:, D * j:D * (j + 1)] for j in range(NPAIR)]

            def emit_qk_chain(j, c, which):
                cs = slice(512 * c, 512 * c + 512)
                ps = pacc.tile([128, 512], f32, tag="acc", name="acc")
                w_all = wq_all if which == 'q' else wk_all
                dst = qT[j] if which == 'q' else kTt[j]
                bias = bq if which == 'q' else bk
                for k in range(KT):
                    nc.tensor.matmul(
                        ps[:, :],
                        lhsT=w_all[:, j * 1024 + k * 128:j * 1024 + (k + 1) * 128],
                        rhs=xt(k, c),
                        start=(k == 0), stop=(k == KT - 1))
                nc.vector.tensor_scalar_add(dst[:, cs], ps[:, :],
                                            bias[:, j:j + 1])

            def emit_v(t):
                ps = pacc.tile([128, 512], f32, tag="acc", name="acc")
                c, i = t // 4, t % 4
                for k in range(KT):
                    nc.tensor.matmul(
                        ps[:, :],
                        lhsT=xt(k, c)[:, 128 * i:128 * i + 128],
                        rhs=wvk[k],
                        start=(k == 0), stop=(k == KT - 1))
                nc.gpsimd.memset(
                    v_all[t].rearrange("p (h e) -> p h e", e=65)[:, :, 64:65],
                    1.0)
                nc.vector.tensor_tensor(
                    v_all[t].rearrange("p (h e) -> p h e", e=65)[:, :, 0:64],
                    ps.rearrange("p (h e) -> p h e", e=64),
                    bv.rearrange("p (h e) -> p h e", e=64),
                    op=OP.add)

            finishers = []

            def emit_attn_block(j, b, fillers, fill_every=1):
                nt = 4 * b + 4
                wvA = pwvp.tile([65, 512], f32, tag="wvA", name="wvA")
                wvB = pwvp.tile([65, 512], f32, tag="wvB", name="wvB")
                live = {}

                def scores(t):
                    off = max(0, 128 * t - 512 * b)
                    qs = slice(512 * b + off, 512 * b + 512)
                    ts = slice(128 * t, 128 * t + 128)
                    ps = pssp.tile([128, 1024], f32, tag="pss", name="pss")
                    nc.tensor.matmul(
                        ps[:, off:512], lhsT=kTt[j][0:64, ts],
                        rhs=qT[j][0:64, qs], start=True, stop=True,
                        tile_position=(0, 0))
                    nc.tensor.matmul(
                        ps[:, 512 + off:1024], lhsT=kTt[j][64:128, ts],
                        rhs=qT[j][64:128, qs], start=True, stop=True,
                        tile_position=(64, 0))
                    live[t] = (ps, attnp.tile([128, 1024], bf16, tag="at",
                                              name="at"), off)

                def expmask(t):
                    ps, at, off = live[t]
                    if off:
                        nc.scalar.activation(
                            at.rearrange("p (h w) -> p h w", h=2)[:, :, off:512],
                            ps.rearrange("p (h w) -> p h w", h=2)[:, :, off:512],
                            AF.Exp, scale=0.125)
                    else:
                        nc.scalar.activation(at[:, :], ps[:, :], AF.Exp,
                                             scale=0.125)
                    if t >= 4 * b:
                        w = 512 - off
                        nc.vector.tensor_tensor(
                            at[:, off:512], at[:, off:512], mask[:, 0:w],
                            op=OP.mult)
                        nc.vector.tensor_tensor(
                            at[:, 512 + off:1024], at[:, 512 + off:1024],
                            mask[:, 0:w], op=OP.mult)

                def wv(t):
                    ps, at, off = live.pop(t)
                    nc.tensor.matmul(
                        wvA[:, off:512],
                        lhsT=v_all[t][:, 130 * j:130 * j + 65],
                        rhs=at[:, off:512],
                        start=(t == 0), stop=(t == nt - 1))
                    nc.tensor.matmul(
                        wvB[:, off:512],
                        lhsT=v_all[t][:, 130 * j + 65:130 * j + 130],
                        rhs=at[:, 512 + off:1024],
                        start=(t == 0), stop=(t == nt - 1))

                # Group-of-2 software pipeline: the two row-tiled scores
                # matmuls of adjacent t-tiles are emitted back-to-back so the
                # PE pays the row-group<->full-array LDWEIGHTS transition
                # (~100ns each way, not hideable across row-group conflicts)
                # once per two tiles instead of once per tile.
                scores(0)
                scores(1)
                expmask(0)
                expmask(1)
                for g in range(1, nt // 2):
                    scores(2 * g)
                    scores(2 * g + 1)
                    wv(2 * g - 2)
                    wv(2 * g - 1)
                    expmask(2 * g)
                    expmask(2 * g + 1)
                    if g == 1 and finishers:
                        finishers.pop(0)()
                    for _ in range(2):
                        if fillers:
                            fillers.pop(0)()
                wv(nt - 2)
                wv(nt - 1)
                while fillers:
                    fillers.pop(0)()

                # drain wv PSUM immediately (frees the banks for the next
                # pair) — wvA on the vector queue, wvB on gpsimd so the two
                # copies land in parallel before the next pair's first wv;
                # then kick off the denominator redistribute.  The DVE
                # reciprocal is deferred into the NEXT pair's attention so
                # it never stalls the in-order DVE queue ahead of that
                # pair's masks.
                bs = slice(512 * b, 512 * b + 512)
                stg = normp.tile([65, 1024], f32, tag="stg", name="stg")
                nc.vector.tensor_copy(stg[:, 0:512], wvA[:, :])
                nc.vector.tensor_copy(stg[:, 512:1024], wvB[:, :])
                sumsq = normp.tile([128, 8], f32, tag="sumsq", name="sumsq")
                nc.sync.dma_start(sumsq[:, :], stg[64:65, :])

                def finish(j=j, b=b, stg=stg, sumsq=sumsq, bs=bs):
                    rq = normp.tile([128, 8], f32, tag="rq", name="rq")
                    nc.vector.reciprocal(rq[:, :], sumsq[:, :])
                    scr = scrp.tile([1, 1024], f32, tag="scr", name="scr")
                    nc.sync.dma_start(scr[:, :], rq[:, :])
                    rsb = normp.tile([64, 1024], f32, tag="rsb", name="rsb")
                    nc.sync.dma_start(rsb[:, :],
                                      scr[0:1, :].broadcast_to((64, 1024)))
                    nc.gpsimd.tensor_tensor(wvT[j][0:64, bs],
                                            stg[0:64, 0:512],
                                            rsb[:, 0:512], op=OP.mult)
                    nc.gpsimd.tensor_tensor(wvT[j][64:128, bs],
                                            stg[0:64, 512:1024],
                                            rsb[:, 512:1024], op=OP.mult)

                def finish_tail(j=j, b=b, stg=stg, bs=bs):
                    # Latency-optimized variant for the very last block: the
                    # DMA-bounce partition broadcast (~10us of chained DMA
                    # completions) is replaced by an fp32 ones-matmul
                    # broadcast of the denominator row into PSUM + a DVE
                    # reciprocal and DVE multiplies (~5us, no DMA).
                    bps = pssp.tile([128, 1024], f32, tag="pss", name="pss")
                    nc.tensor.matmul(bps[:, 0:512], lhsT=ones1[64:65, :],
                                     rhs=stg[64:65, 0:512],
                                     start=True, stop=True)
                    nc.tensor.matmul(bps[:, 512:1024],
                                     lhsT=ones1[64:65, :],
                                     rhs=stg[64:65, 512:1024],
                                     start=True, stop=True)
                    rsb = normp.tile([64, 1024], f32, tag="rsb", name="rsb")
                    nc.vector.reciprocal(rsb[:, :], bps[0:64, :])
                    nc.vector.tensor_tensor(wvT[j][0:64, bs],
                                            stg[0:64, 0:512],
                                            rsb[:, 0:512], op=OP.mult)
                    nc.vector.tensor_tensor(wvT[j][64:128, bs],
                                            stg[0:64, 512:1024],
                                            rsb[:, 512:1024], op=OP.mult)
                finishers.append(finish_tail if (j, b) == (3, 3) else finish)

            def emit_oproj_stile(s):
                ss = slice(128 * s, 128 * s + 128)
                ost = ostp.tile([128, 1024], bf16, tag="ost", name="ost")
                for n in range(2):
                    ns = slice(512 * n, 512 * n + 512)
                    ps = pacc.tile([128, 512], f32, tag="acc", name="acc")
                    for j in range(NPAIR):
                        nc.tensor.matmul(
                            ps[:, :], lhsT=wvT[j][:, ss], rhs=wo[j][:, ns],
                            start=(j == 0), stop=(j == NPAIR - 1))
                    nc.vector.tensor_copy(ost[:, ns], ps[:, :])
                nc.scalar.dma_start(out_d[ss, :], ost[:, :])

            # ---- emission schedule (block-major) ----
            # chains (j, c) are emitted during block c-1; v tiles for block b
            # land as fillers just before/inside the first pair of block b;
            # oproj s-tiles of block b are deferred as late as dependencies
            # allow so the late blocks (largest ACT share) keep the PE fed.
            ch = lambda j, c, w: (lambda: emit_qk_chain(j, c, w))
            vt = lambda t: (lambda: emit_v(t))
            os_ = lambda s: (lambda: emit_oproj_stile(s))

            emit_qk_chain(0, 0, 'q')
            emit_qk_chain(0, 0, 'k')
            for t in range(4):
                emit_v(t)

            # block 0 (nt=4: 3 filler slots per pair)
            emit_attn_block(0, 0, [ch(1, 0, 'q'), ch(1, 0, 'k')])
            emit_attn_block(1, 0, [ch(2, 0, 'q'), ch(2, 0, 'k'),
                                   ch(0, 1, 'q')])
            emit_attn_block(2, 0, [ch(3, 0, 'q'), ch(3, 0, 'k'),
                                   ch(0, 1, 'k')])
            emit_attn_block(3, 0, [vt(4), vt(5), ch(1, 1, 'q')])
            emit_v(6)
            emit_v(7)

            # block 1 (nt=8: 7 slots per pair)
            emit_attn_block(0, 1, [ch(1, 1, 'k'), ch(2, 1, 'q'),
                                   ch(2, 1, 'k')])
            emit_attn_block(1, 1, [ch(3, 1, 'q'), ch(3, 1, 'k'), os_(0)])
            emit_attn_block(2, 1, [ch(0, 2, 'q'), ch(0, 2, 'k'), vt(8),
                                   os_(1)])
            emit_attn_block(3, 1, [ch(1, 2, 'q'), ch(1, 2, 'k'), vt(9),
                                   vt(10)])
            emit_v(11)

            # block 2 (nt=12: 11 slots per pair)
            emit_attn_block(0, 2, [ch(2, 2, 'q'), ch(2, 2, 'k'), os_(2)])
            emit_attn_block(1, 2, [ch(3, 2, 'q'), ch(3, 2, 'k'), os_(3)])
            emit_attn_block(2, 2, [ch(0, 3, 'q'), ch(0, 3, 'k'), vt(12)])
            emit_attn_block(3, 2, [ch(1, 3, 'q'), ch(1, 3, 'k'), vt(13),
                                   vt(14)])
            emit_v(15)

            # block 3 (nt=16: 15 slots per pair)
            emit_attn_block(0, 3, [ch(2, 3, 'q'), ch(2, 3, 'k'), os_(4),
                                   os_(5)])
            emit_attn_block(1, 3, [ch(3, 3, 'q'), ch(3, 3, 'k'), os_(6),
                                   os_(7)])
            emit_attn_block(2, 3, [os_(8), os_(9), os_(10)])
            emit_attn_block(3, 3, [os_(11)])

            # ---- tail ----
            # j=0..2 partial accumulation of oproj s-tiles 12/13 first (they
            # only need pairs 0-2, already normalized), overlapping the
            # (3,3) finisher's DVE chain; then the j=3 closers.  s13's two
            # accumulators live in the halves of the second pss tile (the
            # scores double-buffer is idle by now).
            fin33 = finishers.pop(0)
            assert not finishers
            tail_ps = {}
            for s in (12, 13):
                ss = slice(128 * s, 128 * s + 128)
                if s == 12:
                    ps_pair = [pacc.tile([128, 512], f32, tag="acc",
                                         name="acc") for _ in range(2)]
                else:
                    big = pssp.tile([128, 1024], f32, tag="pss", name="pss")
                    ps_pair = [big[:, 0:512], big[:, 512:1024]]
                for n in range(2):
                    for j in range(3):
                        nc.tensor.matmul(
                            ps_pair[n][:, :], lhsT=wvT[j][:, ss],
                            rhs=wo[j][:, 512 * n:512 * n + 512],
                            start=(j == 0), stop=False)
                tail_ps[s] = ps_pair
            fin33()
            for s in (12, 13):
                ss = slice(128 * s, 128 * s + 128)
                ost = ostp.tile([128, 1024], bf16, tag="ost", name="ost")
                for n in range(2):
                    nc.tensor.matmul(
                        tail_ps[s][n][:, :], lhsT=wvT[3][:, ss],
                        rhs=wo[3][:, 512 * n:512 * n + 512],
                        start=False, stop=True)
                    nc.vector.tensor_copy(ost[:, 512 * n:512 * n + 512],
                                          tail_ps[s][n][:, :])
                nc.scalar.dma_start(out_d[ss, :], ost[:, :])
            emit_oproj_stile(14)
            emit_oproj_stile(15)

    _split_excess_waits(nc, limit=1)
    return nc


def _split_excess_waits(nc, limit=1):
    """This container's walrus encodes at most one sem wait per instruction;
    move excess waits onto standalone EventSemaphore ops just before each
    over-limit instruction (same engine stream, so semantics preserved)."""
    import concourse.mybir as mybir
    n = 0
    for fn in nc.m.functions:
        for bb in fn.blocks:
            new_insts = []
            for inst in bb.instructions:
                si = inst.sync_info
                if si is not None and si.on_wait and len(si.on_wait) > limit:
                    waits = list(si.on_wait)
                    for i, w in enumerate(waits[limit:]):
                        wi = mybir.InstEventSemaphore(
                            name=f"{inst.name}-wsplit{i}", ins=[], outs=[],
                            sync_info=mybir.SyncInfo(on_wait=[w], on_update=[]))
                        wi.engine = inst.engine
                        nc.register_instruction(wi)
                        new_insts.append(wi)
                        n += 1
                    si.on_wait = waits[:limit]
                new_insts.append(inst)
            bb.instructions = new_insts
    return n


def _get_nc():
    if "nc" not in _BUILT:
        _BUILT["nc"] = _build_nc()
    return _BUILT["nc"]


def _prep_core_inputs(x_b, W_q, b_q, W_k, b_k, W_v, b_v, W_o, g):
    """Inputs for one core: batch slice x_b [S, D], head group g (0/1)."""
    import ml_dtypes
    bf16 = ml_dtypes.bfloat16
    hs = slice(g * HPC, (g + 1) * HPC)

    # xT: [p][c][k][sc] = x_b[512c+sc, 128k+p]
    xT = np.ascontiguousarray(
        x_b.reshape(SB, 512, KT, 128).transpose(3, 0, 2, 1)
        .reshape(128, SB * KT * 512)).astype(bf16)

    def arrange_qk(wfull):  # [D, 512] -> [128, NPAIR*KT*128] pair-major
        return np.ascontiguousarray(
            wfull.reshape(KT, 128, NPAIR, 128).transpose(1, 2, 0, 3)
            .reshape(128, NPAIR * KT * 128))

    wq = arrange_qk(W_q[hs].transpose(1, 0, 2).reshape(D, 512)).astype(bf16)
    wk = arrange_qk(W_k[hs].transpose(1, 0, 2).reshape(D, 512)).astype(bf16)
    wv = np.ascontiguousarray(
        W_v[hs].transpose(1, 0, 2).reshape(D, 512)
        .reshape(KT, 128, 512).transpose(1, 0, 2)
        .reshape(128, KT * 512)).astype(bf16)
    wo_t = np.ascontiguousarray(W_o[:, g * 512:(g + 1) * 512].T)  # [512, D]
    wo = np.ascontiguousarray(
        wo_t.reshape(NPAIR, 128, D).transpose(1, 0, 2).reshape(128, NPAIR * D)
    ).astype(bf16)
    bq = np.ascontiguousarray(
        b_q[hs].reshape(NPAIR, 128).T).astype(np.float32)          # [128, 4]
    bk = np.ascontiguousarray(
        b_k[hs].reshape(NPAIR, 128).T).astype(np.float32)
    bv = np.ascontiguousarray(np.broadcast_to(
        b_v[hs].reshape(1, 512), (128, 512))).astype(np.float32)   # [128, 512]

    p = np.arange(128)[:, None]
    cc = np.arange(512)[None, :]
    mask = (cc >= p).astype(bf16)                                  # [128, 512]

    return {"xT": xT, "wq": wq, "wk": wk, "wv": wv, "wo": wo,
            "bq": bq, "bk": bk, "bv": bv, "mask": mask}


def _install_axon_ntff_hook():
    """Register the axon NTFF profiling hook if the environment allows.

    The agent image lacks ``antenv.axon_hooks``; synthesize it and wire the
    ctypes-based profiler from trn_agent_boot so BASS_TRACE=1 yields NTFFs.
    Degrades silently — without it run_bass_kernel_spmd(trace=False) works.
    """
    import sys
    import types
    try:
        import antenv
        if "antenv.axon_hooks" not in sys.modules:
            mod = types.ModuleType("antenv.axon_hooks")
            holder = [None]
            mod.set_axon_ntff_profile_hook = lambda h: holder.__setitem__(0, h)
            mod.get_axon_ntff_profile_hook = lambda: holder[0]
            sys.modules["antenv.axon_hooks"] = mod
            antenv.axon_hooks = mod
        mod = sys.modules["antenv.axon_hooks"]
        if mod.get_axon_ntff_profile_hook() is None:
            from trn_agent_boot.trn_boot import _ntff_profile_via_ctypes
            hook = _ntff_profile_via_ctypes("/opt/axon/libaxon_pjrt.so")
            mod.set_axon_ntff_profile_hook(hook)
        import concourse.bass_utils as bu
        bu.upload_artifacts = lambda d: d  # no S3 in this container
    except Exception:
        pass


def kernel(inputs, W_q, b_q, W_k, b_k, W_v, b_v, W_o, b_o):
    global LAST_RESULTS
    from concourse.bass_utils import run_bass_kernel_spmd
    _install_axon_ntff_hook()

    inputs = np.asarray(inputs, dtype=np.float32)
    args = [np.asarray(a, dtype=np.float32)
            for a in (W_q, b_q, W_k, b_k, W_v, b_v, W_o, b_o)]
    W_q, b_q, W_k, b_k, W_v, b_v, W_o, b_o = args

    nc = _get_nc()
    in_maps = []
    for c in range(NCORES):
        bi, g = c // 2, c % 2
        in_maps.append(_prep_core_inputs(
            inputs[bi], W_q, b_q, W_k, b_k, W_v, b_v, W_o, g))

    res = run_bass_kernel_spmd(nc, in_maps, list(range(NCORES)))
    LAST_RESULTS = res

    out = np.empty((B, S, D), dtype=np.float32)
    for bi in range(B):
        out[bi] = (res.results[2 * bi]["out"].astype(np.float32)
                   + res.results[2 * bi + 1]["out"].astype(np.float32)
                   + b_o[None, :])
    return out


# revision 10
# speedup vs baseline: 1.0632x; 1.0023x over previous
"""Causal multi-head attention TRN2 kernel (8 NeuronCores).

Problem: B=4, S=2048, D=1024, H=16 heads, head_dim=64 (fp32 reference).

Sharding: data-parallel over batch (4) x tensor-parallel over head-groups (2).
Core c handles batch c//2 with heads (c%2)*8 .. (c%2)*8+8 and produces a
partial [S, D] output (its head-group's contribution to the O-projection,
without b_o) in bf16. Host sums the two partials per batch and adds b_o.

Block-major schedule: attention q-blocks are processed in order b=0..3 with
all 4 head-pairs per block, so the O-projection s-tiles, softmax-denominator
normalization chains and output DMAs of block b all retire during block b+1
instead of piling into the kernel tail. PE warm-up matmuls run during the
initial DMA wait so the HAM clock gate opens before real work arrives.
"""

import math

import numpy as np

B, S, D, H = 4, 2048, 1024, 16
HD = D // H        # 64
NCORES = 8
HPC = H // 2       # heads per core: 8
NPAIR = HPC // 2   # head pairs per core: 4
KT = D // 128      # contraction tiles: 8
ST = S // 128      # seq tiles of 128: 16
SB = S // 512      # seq blocks of 512: 4

_BUILT = {}
LAST_RESULTS = None  # BassKernelResults of the most recent run (for test.py)


def _build_nc():
    import concourse.bass as bass
    import concourse.mybir as mybir
    from concourse import tile

    f32 = mybir.dt.float32
    bf16 = mybir.dt.bfloat16
    AF = mybir.ActivationFunctionType
    OP = mybir.AluOpType

    nc = bass.Bass("TRN2", target_bir_lowering=False, debug=False,
                   num_devices=NCORES)

    # DRAM layouts are pre-arranged on the host to match SBUF tiles exactly.
    # xT: col = c*4096 + k*512 + sc  (block-major so block 0 lands first)
    # wq/wk: col = j*1024 + k*128 + e  (pair-major so pair 0 lands first)
    xT_d = nc.dram_tensor("xT", [128, SB * KT * 512], bf16,
                          kind="ExternalInput").ap()
    wq_d = nc.dram_tensor("wq", [128, NPAIR * KT * 128], bf16,
                          kind="ExternalInput").ap()
    wk_d = nc.dram_tensor("wk", [128, NPAIR * KT * 128], bf16,
                          kind="ExternalInput").ap()
    wv_d = nc.dram_tensor("wv", [128, KT * 512], bf16,
                          kind="ExternalInput").ap()
    wo_d = nc.dram_tensor("wo", [128, NPAIR * D], bf16,
                          kind="ExternalInput").ap()
    bq_d = nc.dram_tensor("bq", [128, NPAIR], f32, kind="ExternalInput").ap()
    bk_d = nc.dram_tensor("bk", [128, NPAIR], f32, kind="ExternalInput").ap()
    bv_d = nc.dram_tensor("bv", [128, 512], f32, kind="ExternalInput").ap()
    mask_d = nc.dram_tensor("mask", [128, 512], bf16,
                            kind="ExternalInput").ap()
    out_d = nc.dram_tensor("out", [S, D], bf16, kind="ExternalOutput").ap()

    with tile.TileContext(nc) as tc:
        with tc.tile_pool(name="persist", bufs=1) as pp, \
             tc.tile_pool(name="pacc", bufs=2, space="PSUM") as pacc, \
             tc.tile_pool(name="pss", bufs=2, space="PSUM") as pssp, \
             tc.tile_pool(name="pwv", bufs=1, space="PSUM") as pwvp, \
             tc.tile_pool(name="attn", bufs=6) as attnp, \
             tc.tile_pool(name="norm", bufs=3) as normp, \
             tc.tile_pool(name="ost", bufs=3) as ostp, \
             tc.tile_pool(name="scr", bufs=4, space="DRAM") as scrp:

            xt_all = pp.tile([128, SB * KT * 512], bf16, tag="xt", name="xt")
            wq_all = pp.tile([128, NPAIR * KT * 128], bf16, tag="wq", name="wq")
            wk_all = pp.tile([128, NPAIR * KT * 128], bf16, tag="wk", name="wk")
            wv_all = pp.tile([128, KT * 512], bf16, tag="wv", name="wv")
            wo_all = pp.tile([128, NPAIR * D], bf16, tag="wo", name="wo")
            bq = pp.tile([128, NPAIR], f32, tag="bq", name="bq")
            bk = pp.tile([128, NPAIR], f32, tag="bk", name="bk")
            bv = pp.tile([128, 512], f32, tag="bv", name="bv")
            mask = pp.tile([128, 512], bf16, tag="mask", name="mask")
            qT = [pp.tile([128, S], bf16, tag=f"qT{j}", name=f"qT{j}")
                  for j in range(NPAIR)]
            kTt = [pp.tile([128, S], bf16, tag=f"kT{j}", name=f"kT{j}")
                   for j in range(NPAIR)]
            v_all = [pp.tile([128, 8 * 65], bf16, tag=f"v{t}", name=f"v{t}")
                     for t in range(ST)]
            wvT = [pp.tile([128, S], bf16, tag=f"wvT{j}", name=f"wvT{j}")
                   for j in range(NPAIR)]
            junk = pp.tile([128, 512], bf16, tag="junk", name="junk")

            ones1 = pp.tile([65, 128], f32, tag="ones1", name="ones1")

            # ---- input DMAs ----
            # Three issue streams (scalar/sync/gpsimd) round-robin on fabric
            # bandwidth; order each stream by first-use time.  Each trigger
            # instruction costs ~0.65us on its issuing engine, so chunks are
            # kept big.  Block-major needs all four pairs' W_q/W_k for block
            # 0, so weights stream on scalar while x block 0 splits across
            # sync+gpsimd.
            nc.scalar.dma_start(wq_all[:, 0:1024], wq_d[:, 0:1024])
            nc.scalar.dma_start(wk_all[:, 0:1024], wk_d[:, 0:1024])
            nc.scalar.dma_start(bq, bq_d[:, :])
            nc.scalar.dma_start(bk, bk_d[:, :])
            nc.sync.dma_start(xt_all[:, 0:1024], xT_d[:, 0:1024])
            nc.sync.dma_start(xt_all[:, 1024:2048], xT_d[:, 1024:2048])
            nc.sync.dma_start(mask, mask_d[:, :])
            nc.gpsimd.dma_start(xt_all[:, 2048:3072], xT_d[:, 2048:3072])
            nc.gpsimd.dma_start(xt_all[:, 3072:4096], xT_d[:, 3072:4096])
            nc.scalar.dma_start(wv_all[:, 3072:4096], wv_d[:, 3072:4096])
            nc.sync.dma_start(wv_all[:, 0:1536], wv_d[:, 0:1536])
            nc.gpsimd.dma_start(wv_all[:, 1536:3072], wv_d[:, 1536:3072])
            nc.sync.dma_start(bv, bv_d[:, :])
            # remaining W_q/W_k pairs 1-3 (needed as block-0 fillers)
            nc.scalar.dma_start(wq_all[:, 1024:2560], wq_d[:, 1024:2560])
            nc.scalar.dma_start(wk_all[:, 1024:2560], wk_d[:, 1024:2560])
            nc.scalar.dma_start(wq_all[:, 2560:4096], wq_d[:, 2560:4096])
            nc.scalar.dma_start(wk_all[:, 2560:4096], wk_d[:, 2560:4096])
            # x blocks 1-3
            for c in range(1, SB):
                cs0 = slice(c * 4096, c * 4096 + 2048)
                cs1 = slice(c * 4096 + 2048, (c + 1) * 4096)
                nc.sync.dma_start(xt_all[:, cs0], xT_d[:, cs0])
                nc.gpsimd.dma_start(xt_all[:, cs1], xT_d[:, cs1])
            nc.scalar.dma_start(wo_all, wo_d[:, :])

            # ---- PE warm-up ----
            # ~6us of junk matmuls (no DMA deps; memset on the otherwise-idle
            # vector queue so it issues right after the preamble) flips the
            # HAM clock gate to K=8/8 and keeps the PE busy until the first
            # x/weight chunks land (~12us: ~7.5us fixed preamble before any
            # DMA issues + ~4.5us of transfer).  Output bank is never read.
            nc.vector.memset(junk[:, :], 0.0)
            nc.vector.memset(ones1[:, :], 1.0)
            jps = pacc.tile([128, 512], f32, tag="acc", name="acc")
            for i in range(14):
                nc.tensor.matmul(jps[:, :], lhsT=junk[:, 0:128],
                                 rhs=junk[:, :], start=(i == 0),
                                 stop=(i == 13))
            # prewarm the ACT exp table (first activation triggers a ~2.7us
            # ACT_TABLE_LOAD; don't pay it on the first real softmax tile)
            warm = pp.tile([128, 1], f32, tag="warm", name="warm")
            nc.vector.memset(warm[:, :], 0.0)
            nc.scalar.activation(warm[:, :], warm[:, :], AF.Exp, scale=1.0)

            def xt(k, c):
                return xt_all[:, c * 4096 + k * 512:c * 4096 + (k + 1) * 512]

            wvk = [wv_all[:, 512 * k:512 * (k + 1)] for k in range(KT)]
            wo = [wo_all[:, D * j:D * (j + 1)] for j in range(NPAIR)]

            def emit_qk_chain(j, c, which):
                cs = slice(512 * c, 512 * c + 512)
                ps = pacc.tile([128, 512], f32, tag="acc", name="acc")
                w_all = wq_all if which == 'q' else wk_all
                dst = qT[j] if which == 'q' else kTt[j]
                bias = bq if which == 'q' else bk
                for k in range(KT):
                    nc.tensor.matmul(
                        ps[:, :],
                        lhsT=w_all[:, j * 1024 + k * 128:j * 1024 + (k + 1) * 128],
                        rhs=xt(k, c),
                        start=(k == 0), stop=(k == KT - 1))
                nc.vector.tensor_scalar_add(dst[:, cs], ps[:, :],
                                            bias[:, j:j + 1])

            def emit_v(t):
                ps = pacc.tile([128, 512], f32, tag="acc", name="acc")
                c, i = t // 4, t % 4
                for k in range(KT):
                    nc.tensor.matmul(
                        ps[:, :],
                        lhsT=xt(k, c)[:, 128 * i:128 * i + 128],
                        rhs=wvk[k],
                        start=(k == 0), stop=(k == KT - 1))
                nc.gpsimd.memset(
                    v_all[t].rearrange("p (h e) -> p h e", e=65)[:, :, 64:65],
                    1.0)
                nc.vector.tensor_tensor(
                    v_all[t].rearrange("p (h e) -> p h e", e=65)[:, :, 0:64],
                    ps.rearrange("p (h e) -> p h e", e=64),
                    bv.rearrange("p (h e) -> p h e", e=64),
                    op=OP.add)

            finishers = []

            def emit_attn_block(j, b, fillers, fill_every=1):
                nt = 4 * b + 4
                wvA = pwvp.tile([65, 512], f32, tag="wvA", name="wvA")
                wvB = pwvp.tile([65, 512], f32, tag="wvB", name="wvB")
                live = {}

                def scores(t):
                    off = max(0, 128 * t - 512 * b)
                    qs = slice(512 * b + off, 512 * b + 512)
                    ts = slice(128 * t, 128 * t + 128)
                    ps = pssp.tile([128, 1024], f32, tag="pss", name="pss")
                    nc.tensor.matmul(
                        ps[:, off:512], lhsT=kTt[j][0:64, ts],
                        rhs=qT[j][0:64, qs], start=True, stop=True,
                        tile_position=(0, 0))
                    nc.tensor.matmul(
                        ps[:, 512 + off:1024], lhsT=kTt[j][64:128, ts],
                        rhs=qT[j][64:128, qs], start=True, stop=True,
                        tile_position=(64, 0))
                    live[t] = (ps, attnp.tile([128, 1024], bf16, tag="at",
                                              name="at"), off)

                def expmask(t):
                    ps, at, off = live[t]
                    if off:
                        nc.scalar.activation(
                            at.rearrange("p (h w) -> p h w", h=2)[:, :, off:512],
                            ps.rearrange("p (h w) -> p h w", h=2)[:, :, off:512],
                            AF.Exp, scale=0.125)
                    else:
                        nc.scalar.activation(at[:, :], ps[:, :], AF.Exp,
                                             scale=0.125)
                    if t >= 4 * b:
                        w = 512 - off
                        nc.vector.tensor_tensor(
                            at[:, off:512], at[:, off:512], mask[:, 0:w],
                            op=OP.mult)
                        nc.vector.tensor_tensor(
                            at[:, 512 + off:1024], at[:, 512 + off:1024],
                            mask[:, 0:w], op=OP.mult)

                def wv(t):
                    ps, at, off = live.pop(t)
                    nc.tensor.matmul(
                        wvA[:, off:512],
                        lhsT=v_all[t][:, 130 * j:130 * j + 65],
                        rhs=at[:, off:512],
                        start=(t == 0), stop=(t == nt - 1))
                    nc.tensor.matmul(
                        wvB[:, off:512],
                        lhsT=v_all[t][:, 130 * j + 65:130 * j + 130],
                        rhs=at[:, 512 + off:1024],
                        start=(t == 0), stop=(t == nt - 1))

                # Group-of-2 software pipeline: the two row-tiled scores
                # matmuls of adjacent t-tiles are emitted back-to-back so the
                # PE pays the row-group<->full-array LDWEIGHTS transition
                # (~100ns each way, not hideable across row-group conflicts)
                # once per two tiles instead of once per tile.
                scores(0)
                scores(1)
                expmask(0)
                expmask(1)
                for g in range(1, nt // 2):
                    scores(2 * g)
                    scores(2 * g + 1)
                    wv(2 * g - 2)
                    wv(2 * g - 1)
                    expmask(2 * g)
                    expmask(2 * g + 1)
                    if g == 1 and finishers:
                        finishers.pop(0)()
                    for _ in range(2):
                        if fillers:
                            fillers.pop(0)()
                wv(nt - 2)
                wv(nt - 1)
                while fillers:
                    fillers.pop(0)()

                # drain wv PSUM immediately (frees the banks for the next
                # pair) — wvA on the vector queue, wvB on gpsimd so the two
                # copies land in parallel before the next pair's first wv;
                # then kick off the denominator redistribute.  The DVE
                # reciprocal is deferred into the NEXT pair's attention so
                # it never stalls the in-order DVE queue ahead of that
                # pair's masks.
                bs = slice(512 * b, 512 * b + 512)
                stg = normp.tile([65, 1024], f32, tag="stg", name="stg")
                if (j, b) == (3, 3):
                    nc.scalar.copy(stg[:, 0:512], wvA[:, :])
                    nc.scalar.copy(stg[:, 512:1024], wvB[:, :])
                else:
                    nc.vector.tensor_copy(stg[:, 0:512], wvA[:, :])
                    nc.vector.tensor_copy(stg[:, 512:1024], wvB[:, :])
                sumsq = normp.tile([128, 8], f32, tag="sumsq", name="sumsq")
                nc.sync.dma_start(sumsq[:, :], stg[64:65, :])

                def finish(j=j, b=b, stg=stg, sumsq=sumsq, bs=bs):
                    rq = normp.tile([128, 8], f32, tag="rq", name="rq")
                    nc.vector.reciprocal(rq[:, :], sumsq[:, :])
                    scr = scrp.tile([1, 1024], f32, tag="scr", name="scr")
                    nc.sync.dma_start(scr[:, :], rq[:, :])
                    rsb = normp.tile([64, 1024], f32, tag="rsb", name="rsb")
                    nc.sync.dma_start(rsb[:, :],
                                      scr[0:1, :].broadcast_to((64, 1024)))
                    nc.gpsimd.tensor_tensor(wvT[j][0:64, bs],
                                            stg[0:64, 0:512],
                                            rsb[:, 0:512], op=OP.mult)
                    nc.gpsimd.tensor_tensor(wvT[j][64:128, bs],
                                            stg[0:64, 512:1024],
                                            rsb[:, 512:1024], op=OP.mult)

                def finish_tail(j=j, b=b, stg=stg, bs=bs):
                    # Latency-optimized variant for the very last block: the
                    # DMA-bounce partition broadcast (~10us of chained DMA
                    # completions) is replaced by an fp32 ones-matmul
                    # broadcast of the denominator row into PSUM + a DVE
                    # reciprocal and DVE multiplies (~5us, no DMA).
                    bps = pssp.tile([128, 1024], f32, tag="pss", name="pss")
                    nc.tensor.matmul(bps[:, 0:512], lhsT=ones1[64:65, :],
                                     rhs=stg[64:65, 0:512],
                                     start=True, stop=True)
                    nc.tensor.matmul(bps[:, 512:1024],
                                     lhsT=ones1[64:65, :],
                                     rhs=stg[64:65, 512:1024],
                                     start=True, stop=True)
                    rsb = normp.tile([64, 1024], f32, tag="rsb", name="rsb")
                    nc.vector.reciprocal(rsb[:, :], bps[0:64, :])
                    nc.vector.tensor_tensor(wvT[j][0:64, bs],
                                            stg[0:64, 0:512],
                                            rsb[:, 0:512], op=OP.mult)
                    nc.vector.tensor_tensor(wvT[j][64:128, bs],
                                            stg[0:64, 512:1024],
                                            rsb[:, 512:1024], op=OP.mult)
                finishers.append(finish_tail if (j, b) == (3, 3) else finish)

            def emit_oproj_stile(s):
                ss = slice(128 * s, 128 * s + 128)
                ost = ostp.tile([128, 1024], bf16, tag="ost", name="ost")
                for n in range(2):
                    ns = slice(512 * n, 512 * n + 512)
                    ps = pacc.tile([128, 512], f32, tag="acc", name="acc")
                    for j in range(NPAIR):
                        nc.tensor.matmul(
                            ps[:, :], lhsT=wvT[j][:, ss], rhs=wo[j][:, ns],
                            start=(j == 0), stop=(j == NPAIR - 1))
                    nc.vector.tensor_copy(ost[:, ns], ps[:, :])
                nc.sync.dma_start(out_d[ss, :], ost[:, :])

            # ---- emission schedule (block-major) ----
            # chains (j, c) are emitted during block c-1; v tiles for block b
            # land as fillers just before/inside the first pair of block b;
            # oproj s-tiles of block b are deferred as late as dependencies
            # allow so the late blocks (largest ACT share) keep the PE fed.
            ch = lambda j, c, w: (lambda: emit_qk_chain(j, c, w))
            vt = lambda t: (lambda: emit_v(t))
            os_ = lambda s: (lambda: emit_oproj_stile(s))

            emit_qk_chain(0, 0, 'q')
            emit_qk_chain(0, 0, 'k')
            for t in range(4):
                emit_v(t)

            # block 0 (nt=4: 3 filler slots per pair)
            emit_attn_block(0, 0, [ch(1, 0, 'q'), ch(1, 0, 'k')])
            emit_attn_block(1, 0, [ch(2, 0, 'q'), ch(2, 0, 'k'),
                                   ch(0, 1, 'q')])
            emit_attn_block(2, 0, [ch(3, 0, 'q'), ch(3, 0, 'k'),
                                   ch(0, 1, 'k')])
            emit_attn_block(3, 0, [vt(4), vt(5), ch(1, 1, 'q')])
            emit_v(6)
            emit_v(7)

            # block 1 (nt=8: 7 slots per pair)
            emit_attn_block(0, 1, [ch(1, 1, 'k'), ch(2, 1, 'q'),
                                   ch(2, 1, 'k')])
            emit_attn_block(1, 1, [ch(3, 1, 'q'), ch(3, 1, 'k')])
            emit_attn_block(2, 1, [ch(0, 2, 'q'), ch(0, 2, 'k'), vt(8)])
            emit_attn_block(3, 1, [ch(1, 2, 'q'), ch(1, 2, 'k'), vt(9),
                                   vt(10)])
            emit_v(11)

            # block 2 (nt=12: 11 slots per pair)
            emit_attn_block(0, 2, [ch(2, 2, 'q'), ch(2, 2, 'k')])
            emit_attn_block(1, 2, [ch(3, 2, 'q'), ch(3, 2, 'k')])
            emit_attn_block(2, 2, [ch(0, 3, 'q'), ch(0, 3, 'k'), vt(12)])
            emit_attn_block(3, 2, [ch(1, 3, 'q'), ch(1, 3, 'k'), vt(13),
                                   vt(14)])
            emit_v(15)

            # block 3 (nt=16: 15 slots per pair)
            emit_attn_block(0, 3, [ch(2, 3, 'q'), ch(2, 3, 'k'), os_(0),
                                   os_(1), os_(2), os_(3)])
            emit_attn_block(1, 3, [ch(3, 3, 'q'), ch(3, 3, 'k'), os_(4),
                                   os_(5), os_(6), os_(7)])
            emit_attn_block(2, 3, [os_(8), os_(9), os_(10)])
            emit_attn_block(3, 3, [os_(11)])

            # ---- tail ----
            # j=0..2 partial accumulation of oproj s-tiles 12/13 first (they
            # only need pairs 0-2, already normalized), overlapping the
            # (3,3) finisher's DVE chain; then the j=3 closers.  s13's two
            # accumulators live in the halves of the second pss tile (the
            # scores double-buffer is idle by now).
            fin33 = finishers.pop(0)
            assert not finishers
            tail_ps = {}
            for s in (12, 13):
                ss = slice(128 * s, 128 * s + 128)
                if s == 12:
                    ps_pair = [pacc.tile([128, 512], f32, tag="acc",
                                         name="acc") for _ in range(2)]
                else:
                    big = pssp.tile([128, 1024], f32, tag="pss", name="pss")
                    ps_pair = [big[:, 0:512], big[:, 512:1024]]
                for n in range(2):
                    for j in range(3):
                        nc.tensor.matmul(
                            ps_pair[n][:, :], lhsT=wvT[j][:, ss],
                            rhs=wo[j][:, 512 * n:512 * n + 512],
                            start=(j == 0), stop=False)
                tail_ps[s] = ps_pair
            fin33()
            for s in (12, 13):
                ss = slice(128 * s, 128 * s + 128)
                ost = ostp.tile([128, 1024], bf16, tag="ost", name="ost")
                for n in range(2):
                    nc.tensor.matmul(
                        tail_ps[s][n][:, :], lhsT=wvT[3][:, ss],
                        rhs=wo[3][:, 512 * n:512 * n + 512],
                        start=False, stop=True)
                    nc.vector.tensor_copy(ost[:, 512 * n:512 * n + 512],
                                          tail_ps[s][n][:, :])
                nc.sync.dma_start(out_d[ss, :], ost[:, :])
            emit_oproj_stile(14)
            emit_oproj_stile(15)

    _split_excess_waits(nc, limit=1)
    return nc


def _split_excess_waits(nc, limit=1):
    """This container's walrus encodes at most one sem wait per instruction;
    move excess waits onto standalone EventSemaphore ops just before each
    over-limit instruction (same engine stream, so semantics preserved)."""
    import concourse.mybir as mybir
    n = 0
    for fn in nc.m.functions:
        for bb in fn.blocks:
            new_insts = []
            for inst in bb.instructions:
                si = inst.sync_info
                if si is not None and si.on_wait and len(si.on_wait) > limit:
                    waits = list(si.on_wait)
                    for i, w in enumerate(waits[limit:]):
                        wi = mybir.InstEventSemaphore(
                            name=f"{inst.name}-wsplit{i}", ins=[], outs=[],
                            sync_info=mybir.SyncInfo(on_wait=[w], on_update=[]))
                        wi.engine = inst.engine
                        nc.register_instruction(wi)
                        new_insts.append(wi)
                        n += 1
                    si.on_wait = waits[:limit]
                new_insts.append(inst)
            bb.instructions = new_insts
    return n


def _get_nc():
    if "nc" not in _BUILT:
        _BUILT["nc"] = _build_nc()
    return _BUILT["nc"]


def _prep_core_inputs(x_b, W_q, b_q, W_k, b_k, W_v, b_v, W_o, g):
    """Inputs for one core: batch slice x_b [S, D], head group g (0/1)."""
    import ml_dtypes
    bf16 = ml_dtypes.bfloat16
    hs = slice(g * HPC, (g + 1) * HPC)

    # xT: [p][c][k][sc] = x_b[512c+sc, 128k+p]
    xT = np.ascontiguousarray(
        x_b.reshape(SB, 512, KT, 128).transpose(3, 0, 2, 1)
        .reshape(128, SB * KT * 512)).astype(bf16)

    def arrange_qk(wfull):  # [D, 512] -> [128, NPAIR*KT*128] pair-major
        return np.ascontiguousarray(
            wfull.reshape(KT, 128, NPAIR, 128).transpose(1, 2, 0, 3)
            .reshape(128, NPAIR * KT * 128))

    wq = arrange_qk(W_q[hs].transpose(1, 0, 2).reshape(D, 512)).astype(bf16)
    wk = arrange_qk(W_k[hs].transpose(1, 0, 2).reshape(D, 512)).astype(bf16)
    wv = np.ascontiguousarray(
        W_v[hs].transpose(1, 0, 2).reshape(D, 512)
        .reshape(KT, 128, 512).transpose(1, 0, 2)
        .reshape(128, KT * 512)).astype(bf16)
    wo_t = np.ascontiguousarray(W_o[:, g * 512:(g + 1) * 512].T)  # [512, D]
    wo = np.ascontiguousarray(
        wo_t.reshape(NPAIR, 128, D).transpose(1, 0, 2).reshape(128, NPAIR * D)
    ).astype(bf16)
    bq = np.ascontiguousarray(
        b_q[hs].reshape(NPAIR, 128).T).astype(np.float32)          # [128, 4]
    bk = np.ascontiguousarray(
        b_k[hs].reshape(NPAIR, 128).T).astype(np.float32)
    bv = np.ascontiguousarray(np.broadcast_to(
        b_v[hs].reshape(1, 512), (128, 512))).astype(np.float32)   # [128, 512]

    p = np.arange(128)[:, None]
    cc = np.arange(512)[None, :]
    mask = (cc >= p).astype(bf16)                                  # [128, 512]

    return {"xT": xT, "wq": wq, "wk": wk, "wv": wv, "wo": wo,
            "bq": bq, "bk": bk, "bv": bv, "mask": mask}


def _install_axon_ntff_hook():
    """Register the axon NTFF profiling hook if the environment allows.

    The agent image lacks ``antenv.axon_hooks``; synthesize it and wire the
    ctypes-based profiler from trn_agent_boot so BASS_TRACE=1 yields NTFFs.
    Degrades silently — without it run_bass_kernel_spmd(trace=False) works.
    """
    import sys
    import types
    try:
        import antenv
        if "antenv.axon_hooks" not in sys.modules:
            mod = types.ModuleType("antenv.axon_hooks")
            holder = [None]
            mod.set_axon_ntff_profile_hook = lambda h: holder.__setitem__(0, h)
            mod.get_axon_ntff_profile_hook = lambda: holder[0]
            sys.modules["antenv.axon_hooks"] = mod
            antenv.axon_hooks = mod
        mod = sys.modules["antenv.axon_hooks"]
        if mod.get_axon_ntff_profile_hook() is None:
            from trn_agent_boot.trn_boot import _ntff_profile_via_ctypes
            hook = _ntff_profile_via_ctypes("/opt/axon/libaxon_pjrt.so")
            mod.set_axon_ntff_profile_hook(hook)
        import concourse.bass_utils as bu
        bu.upload_artifacts = lambda d: d  # no S3 in this container
    except Exception:
        pass


def kernel(inputs, W_q, b_q, W_k, b_k, W_v, b_v, W_o, b_o):
    global LAST_RESULTS
    from concourse.bass_utils import run_bass_kernel_spmd
    _install_axon_ntff_hook()

    inputs = np.asarray(inputs, dtype=np.float32)
    args = [np.asarray(a, dtype=np.float32)
            for a in (W_q, b_q, W_k, b_k, W_v, b_v, W_o, b_o)]
    W_q, b_q, W_k, b_k, W_v, b_v, W_o, b_o = args

    nc = _get_nc()
    in_maps = []
    for c in range(NCORES):
        bi, g = c // 2, c % 2
        in_maps.append(_prep_core_inputs(
            inputs[bi], W_q, b_q, W_k, b_k, W_v, b_v, W_o, g))

    res = run_bass_kernel_spmd(nc, in_maps, list(range(NCORES)))
    LAST_RESULTS = res

    out = np.empty((B, S, D), dtype=np.float32)
    for bi in range(B):
        out[bi] = (res.results[2 * bi]["out"].astype(np.float32)
                   + res.results[2 * bi + 1]["out"].astype(np.float32)
                   + b_o[None, :])
    return out
